# revision 13
# baseline (speedup 1.0000x reference)
"""CausalWanSelfAttention TRN2 kernel: 8-core SPMD via bass/tile.

Sharding: head-tensor-parallel with query-split for the 4 "extra" heads.
 - core c slot0: head c (0..7), all S=3120 queries, full KV.
 - core c slot1: head 8+(c%4), query-half (c//4), full KV (KV dup x2).
RMS-norm coupling across heads handled by a tiny ssq AllReduce.
Output projection partials combined by ReduceScatter (global 8-core for
slot0, two 4-core groups for slot1 halves), concatenated on host.

All matmuls f32r (QK, projections, o-proj) except softmax*V which is bf16.
q/k feature order is permuted (evens-then-odds within each head) so RoPE
becomes partition-contiguous half-block ops; scores are invariant.
"""

import math
import sys

import numpy as np

sys.path.insert(0, "/opt/trn_rl_repo")

import ml_dtypes  # noqa: E402

NUM_HEADS = 12
DIM = 1536
HD = 128
S = 3120
HALF = 1560
CACHED = 2512
KEYS = CACHED + S  # 5632
MAX_ATTN = 5632
EPS = 1e-6
CT = DIM // 128  # 12
N_CORES = 8
SCALE = 1.0 / math.sqrt(HD)
OUT_ROWS = S // N_CORES  # 390

# key tiles: 19 full cached, 1x80 cached, 24 full new, 1x48 new
KEY_TILES = (
    [(i * 128, 128) for i in range(19)]
    + [(2432, 80)]
    + [(CACHED + i * 128, 128) for i in range(24)]
    + [(CACHED + 3072, 48)]
)
N_KT = len(KEY_TILES)  # 45

EXP_GROUPS = (
    [(i, i + 1) for i in range(0, 18, 2)]
    + [(18,), (19,)]
    + [(i, i + 1) for i in range(20, 44, 2)]
    + [(44,)]
)

S_CHUNKS = [(i * 256, 256) for i in range(12)] + [(3072, 48)]
H_CHUNKS = [(i * 256, 256) for i in range(6)] + [(1536, 24)]
Q_CHUNKS0 = [(i * 512, 512) for i in range(6)] + [(3072, 48)]
Q_CHUNKS1 = [(i * 512, 512) for i in range(3)] + [(1536, 24)]

_BUILD_CACHE = {}


def _build():
    if "nc" in _BUILD_CACHE:
        return _BUILD_CACHE["nc"]

    import concourse.mybir as mybir
    import concourse.tile as tile
    from concourse import bacc
    from concourse.masks import make_identity

    dt = mybir.dt
    AF = mybir.ActivationFunctionType
    OP = mybir.AluOpType

    nc = bacc.Bacc("TRN2", num_devices=N_CORES, debug=False)

    def din(name, shape, dtype=dt.float32):
        return nc.dram_tensor(name, shape, dtype, kind="ExternalInput").ap()

    xT = din("xT", [DIM, S])
    xu = din("xu", [DIM, HALF])
    w_in = {n: din(n, [DIM, 128]) for n in ("wq0", "wq1", "wk0", "wk1", "wv0", "wv1")}
    b_in = {n: din(n, [128, 1]) for n in ("bq0", "bq1", "bk0", "bk1", "bv0", "bv1")}
    g_in = {
        n: din(n, [128, 1])
        for n in ("gq0", "gq0s", "gq1", "gq1s", "gk0", "gk0s", "gk1", "gk1s")
    }
    cosq = din("cosq", [128, S])
    sinq = din("sinq", [128, S])
    cosq1 = din("cosq1", [128, HALF])
    sinq1 = din("sinq1", [128, HALF])
    kc_in = [din("kc0", [128, CACHED]), din("kc1", [128, CACHED])]
    vc_in = [din("vc0", [CACHED, 128], dt.bfloat16), din("vc1", [CACHED, 128], dt.bfloat16)]
    wo_in = [din("wo0", [128, DIM]), din("wo1", [128, DIM])]
    bo128 = din("bo128", [128, DIM])
    masks = {n: din(n, [1, 1]) for n in ("mk1", "mq3", "mq4", "rnsel", "rnsel2")}
    out_ap = nc.dram_tensor("out", [OUT_ROWS, DIM], dt.float32,
                            kind="ExternalOutput").ap()

    with tile.TileContext(nc) as tc:
        with (
            tc.tile_pool(name="persist", bufs=1) as persist,
            tc.tile_pool(name="dram", bufs=1, space="DRAM") as dram,
        ):
            # ---------- persistent tiles ----------
            kT = [persist.tile([128, KEYS], dt.float32r, tag=f"kT{i}", name=f"kT{i}")
                  for i in range(2)]
            qT0 = persist.tile([128, S], dt.float32r, tag="qT0")
            qT1 = persist.tile([128, HALF], dt.float32r, tag="qT1")
            v_sb = [persist.tile([128, N_KT, 128], dt.bfloat16, tag=f"v{i}", name=f"v{i}")
                    for i in range(2)]
            wo_r = [persist.tile([128, DIM], dt.float32r, tag=f"wo{i}", name=f"wo{i}")
                    for i in range(2)]
            ones_r = persist.tile([128, 1], dt.float32r, tag="ones_r")
            ones_b = persist.tile([128, 1], dt.bfloat16, tag="ones_b")
            ident = persist.tile([128, 128], dt.float32, tag="ident")
            m_ap = {}
            for n in masks:
                t = persist.tile([1, 1], dt.float32, tag=f"m_{n}", name=f"m_{n}")
                nc.sync.dma_start(out=t, in_=masks[n])
                m_ap[n] = t

            make_identity(nc, ident)
            tmp1 = persist.tile([128, 1], dt.float32, tag="tmp1")
            nc.vector.memset(tmp1, 1.0)
            nc.vector.tensor_copy(ones_r, tmp1)
            nc.vector.memset(ones_b, 1.0)

            # cached V -> v tiles (early; overlaps with proj compute)
            for i in range(2):
                nc.sync.dma_start(
                    out=v_sb[i][:, 0:19, :],
                    in_=vc_in[i][0:2432, :].rearrange("(t p) d -> p t d", p=128),
                )
                nc.sync.dma_start(out=v_sb[i][0:80, 19, :], in_=vc_in[i][2432:2512, :])

            ssq_in = dram.tile([8, S], dt.float32, tag="ssq_in")
            ssq_out = dram.tile([8, S], dt.float32, tag="ssq_out")

            # ---------- phase 1: projections + rope ----------
            with (
                tc.tile_pool(name="bgp", bufs=1) as bgp,
                tc.tile_pool(name="xstage", bufs=2) as xs,
                tc.tile_pool(name="xr", bufs=2) as xrp,
                tc.tile_pool(name="praw", bufs=3) as prp,
                tc.tile_pool(name="ptmp", bufs=2) as ptp,
                tc.tile_pool(name="ssqs", bufs=3) as ssqsb,
                tc.tile_pool(name="p1ps", bufs=4, space="PSUM") as p1ps,
                tc.tile_pool(name="ssqps", bufs=2, space="PSUM") as ssqps,
                tc.tile_pool(name="tps", bufs=2, space="PSUM") as tps,
            ):
                b_ap = {}
                for n, src in list(b_in.items()) + list(g_in.items()):
                    t = bgp.tile([128, 1], dt.float32, tag=f"b_{n}", name=f"b_{n}")
                    nc.sync.dma_start(out=t, in_=src)
                    b_ap[n] = t

                def proj_chunk(xsrc, soff, ssz, specs, ctab_d, stab_d):
                    raw = xs.tile([128, CT, 256], dt.float32, tag="x")
                    nc.sync.dma_start(
                        out=raw[:, :, 0:ssz],
                        in_=xsrc.rearrange("(ct p) s -> p ct s", p=128)[
                            :, :, soff:soff + ssz],
                    )
                    ctab = ptp.tile([128, 256], dt.float32, tag="ctab")
                    stab = ptp.tile([128, 256], dt.float32, tag="stab")
                    nc.sync.dma_start(out=ctab[:, 0:ssz],
                                      in_=ctab_d[:, soff:soff + ssz])
                    nc.sync.dma_start(out=stab[:, 0:ssz],
                                      in_=stab_d[:, soff:soff + ssz])
                    xr = xrp.tile([128, CT, 256], dt.float32r, tag="xr")
                    nc.scalar.activation(out=xr[:, :, 0:ssz], in_=raw[:, :, 0:ssz],
                                         func=AF.Copy, bias=0.0, scale=1.0)
                    for spec in specs:
                        kind, wname, bname = spec[:3]
                        ps = p1ps.tile([128, 256], dt.float32, tag="projps")
                        for ct in range(CT):
                            nc.tensor.matmul(ps[:, 0:ssz], w_r[wname][:, ct],
                                             xr[:, ct, 0:ssz],
                                             start=(ct == 0), stop=(ct == CT - 1))
                        pr = prp.tile([128, 256], dt.float32, tag="praw")
                        nc.vector.tensor_scalar(
                            out=pr[:, 0:ssz], in0=ps[:, 0:ssz],
                            scalar1=b_ap[bname], scalar2=None, op0=OP.add)
                        if kind == "v":
                            vi = spec[3]
                            for bo_ in range(0, ssz, 128):
                                bsz = min(128, ssz - bo_)
                                ptile = tps.tile([128, 128], dt.float32, tag="tp")
                                nc.tensor.transpose(
                                    ptile[0:bsz, :], pr[:, bo_:bo_ + bsz], ident)
                                ti = 20 + (soff + bo_) // 128
                                nc.vector.tensor_copy(
                                    v_sb[vi][0:bsz, ti, :], ptile[0:bsz, :])
                        else:
                            rows, gname, gsname, dest = spec[3:]
                            sq = ptp.tile([128, 256], dt.float32r, tag="sq")
                            nc.vector.tensor_mul(sq[:, 0:ssz], pr[:, 0:ssz],
                                                 pr[:, 0:ssz])
                            sps = ssqps.tile([1, 256], dt.float32, tag="ssqps")
                            nc.tensor.matmul(sps[0:1, 0:ssz], ones_r, sq[:, 0:ssz],
                                             start=True, stop=True)
                            for row, mask in rows:
                                srow = ssqsb.tile([1, 256], dt.float32, tag="ssq")
                                nc.vector.tensor_scalar(
                                    out=srow[0:1, 0:ssz],
                                    in0=sps[0:1, 0:ssz],
                                    scalar1=(m_ap[mask] if mask else 1.0),
                                    scalar2=None, op0=OP.mult)
                                nc.sync.dma_start(
                                    out=ssq_in[row:row + 1, soff:soff + ssz],
                                    in_=srow[0:1, 0:ssz])
                            # rope: dest = (pr*g) . cos + (swap(pr)*gs) . sin
                            sh = ptp.tile([128, 256], dt.float32, tag="sh")
                            nc.sync.dma_start(out=sh[0:64, 0:ssz],
                                              in_=pr[64:128, 0:ssz])
                            nc.sync.dma_start(out=sh[64:128, 0:ssz],
                                              in_=pr[0:64, 0:ssz])
                            tc_ = ptp.tile([128, 256], dt.float32, tag="tcos")
                            nc.vector.scalar_tensor_tensor(
                                out=tc_[:, 0:ssz], in0=pr[:, 0:ssz],
                                scalar=b_ap[gname],
                                in1=ctab[:, 0:ssz],
                                op0=OP.mult, op1=OP.mult)
                            ts_ = ptp.tile([128, 256], dt.float32, tag="tsin")
                            nc.vector.scalar_tensor_tensor(
                                out=ts_[:, 0:ssz], in0=sh[:, 0:ssz],
                                scalar=b_ap[gsname],
                                in1=stab[:, 0:ssz],
                                op0=OP.mult, op1=OP.mult)
                            nc.vector.tensor_add(dest[:, soff:soff + ssz],
                                                 tc_[:, 0:ssz], ts_[:, 0:ssz])

                # --- main S loop (slot0 q/k/v + slot1 k/v) ---
                with (
                    tc.tile_pool(name="wts", bufs=1) as wpool,
                    tc.tile_pool(name="wstage", bufs=1) as ws,
                ):
                    w_r = {}
                    for n in ("wq0", "wk0", "wv0", "wk1", "wv1"):
                        rawt = ws.tile([128, CT, 128], dt.float32, tag="wstage")
                        nc.sync.dma_start(
                            out=rawt, in_=w_in[n].rearrange("(ct p) f -> p ct f", p=128))
                        wr = wpool.tile([128, CT, 128], dt.float32r, tag=f"w_{n}")
                        nc.vector.tensor_copy(wr, rawt)
                        w_r[n] = wr

                    for soff, ssz in S_CHUNKS:
                        proj_chunk(
                            xT, soff, ssz,
                            [
                                ("qk", "wq0", "bq0", [(2, None)], "gq0", "gq0s", qT0),
                                ("qk", "wk0", "bk0", [(0, None)], "gk0", "gk0s",
                                 kT[0][:, CACHED:]),
                                ("qk", "wk1", "bk1", [(1, "mk1")], "gk1", "gk1s",
                                 kT[1][:, CACHED:]),
                            ],
                            cosq, sinq,
                        )

                # --- slot1 q loop ---
                with (
                    tc.tile_pool(name="wts1", bufs=1) as wpool1,
                    tc.tile_pool(name="wstage1", bufs=1) as ws1,
                ):
                    rawt = ws1.tile([128, CT, 128], dt.float32, tag="wstage")
                    nc.sync.dma_start(
                        out=rawt, in_=w_in["wq1"].rearrange("(ct p) f -> p ct f", p=128))
                    wq1r = wpool1.tile([128, CT, 128], dt.float32r, tag="w_wq1")
                    nc.vector.tensor_copy(wq1r, rawt)
                    w_r["wq1"] = wq1r
                    for soff, ssz in H_CHUNKS:
                        proj_chunk(
                            xu, soff, ssz,
                            [("qk", "wq1", "bq1", [(3, "mq3"), (4, "mq4")],
                              "gq1", "gq1s", qT1)],
                            cosq1, sinq1,
                        )

            # ---------- ssq AllReduce (issued before V so it overlaps) ----------
            nc.gpsimd.collective_compute(
                "AllReduce", OP.add,
                replica_groups=[list(range(N_CORES))],
                ins=[ssq_in.opt()], outs=[ssq_out.opt()],
            )

            # ---------- V projections (overlap the AllReduce) ----------
            with (
                tc.tile_pool(name="xstage2", bufs=2) as xs2,
                tc.tile_pool(name="xr2", bufs=2) as xrp2,
                tc.tile_pool(name="praw2", bufs=3) as prp2,
                tc.tile_pool(name="wtsv", bufs=1) as wpoolv,
                tc.tile_pool(name="wstagev", bufs=1) as wsv,
                tc.tile_pool(name="p1ps2", bufs=4, space="PSUM") as p1ps2,
                tc.tile_pool(name="tps2", bufs=2, space="PSUM") as tps2,
            ):
                w_rv = {}
                bv_ap = {}
                for n in ("bv0", "bv1"):
                    t = wpoolv.tile([128, 1], dt.float32, tag=f"bv_{n}", name=f"bv_{n}")
                    nc.sync.dma_start(out=t, in_=b_in[n])
                    bv_ap[n] = t
                for n in ("wv0", "wv1"):
                    rawt = wsv.tile([128, CT, 128], dt.float32, tag="wstage")
                    nc.sync.dma_start(
                        out=rawt, in_=w_in[n].rearrange("(ct p) f -> p ct f", p=128))
                    wr = wpoolv.tile([128, CT, 128], dt.float32r, tag=f"wv_{n}")
                    nc.vector.tensor_copy(wr, rawt)
                    w_rv[n] = wr
                for soff, ssz in S_CHUNKS:
                    raw = xs2.tile([128, CT, 256], dt.float32, tag="x")
                    nc.sync.dma_start(
                        out=raw[:, :, 0:ssz],
                        in_=xT.rearrange("(ct p) s -> p ct s", p=128)[
                            :, :, soff:soff + ssz],
                    )
                    xr = xrp2.tile([128, CT, 256], dt.float32r, tag="xr")
                    nc.scalar.activation(out=xr[:, :, 0:ssz], in_=raw[:, :, 0:ssz],
                                         func=AF.Copy, bias=0.0, scale=1.0)
                    for vi, wname, bname in ((0, "wv0", "bv0"), (1, "wv1", "bv1")):
                        ps = p1ps2.tile([128, 256], dt.float32, tag="projps")
                        for ct in range(CT):
                            nc.tensor.matmul(ps[:, 0:ssz], w_rv[wname][:, ct],
                                             xr[:, ct, 0:ssz],
                                             start=(ct == 0), stop=(ct == CT - 1))
                        pr = prp2.tile([128, 256], dt.float32, tag="praw")
                        nc.vector.tensor_scalar(
                            out=pr[:, 0:ssz], in0=ps[:, 0:ssz],
                            scalar1=bv_ap[bname], scalar2=None, op0=OP.add)
                        for bo_ in range(0, ssz, 128):
                            bsz = min(128, ssz - bo_)
                            ptile = tps2.tile([128, 128], dt.float32, tag="tp")
                            nc.tensor.transpose(
                                ptile[0:bsz, :], pr[:, bo_:bo_ + bsz], ident)
                            ti = 20 + (soff + bo_) // 128
                            nc.vector.tensor_copy(
                                v_sb[vi][0:bsz, ti, :], ptile[0:bsz, :])

            # ---------- cached K load (round to f32r) ----------
            with tc.tile_pool(name="kcstage", bufs=2) as kcs:
                for i in range(2):
                    t = kcs.tile([128, CACHED], dt.float32, tag="kc")
                    nc.sync.dma_start(out=t, in_=kc_in[i])
                    nc.vector.tensor_copy(kT[i][:, 0:CACHED], t)
                # round wo while we're at it
                for i in range(2):
                    t2 = kcs.tile([128, DIM], dt.float32, tag="wos")
                    nc.sync.dma_start(out=t2, in_=wo_in[i])
                    nc.vector.tensor_copy(wo_r[i], t2)
            with tc.tile_pool(name="normp", bufs=1) as npo:
                eps_t = npo.tile([1, 1], dt.float32, tag="eps")
                nc.vector.memset(eps_t, EPS)
                rn_d = dram.tile([3, S], dt.float32, tag="rn_d")
                sc1 = npo.tile([1, S], dt.float32, tag="sc1")
                sc2 = npo.tile([1, S], dt.float32, tag="sc2")
                sc3 = npo.tile([1, S], dt.float32, tag="sc3")
                bb = npo.tile([128, S], dt.float32, tag="bb")
                eps128 = npo.tile([128, 1], dt.float32, tag="eps128")
                nc.vector.memset(eps128, EPS)
                sq_d = dram.tile([2, S], dt.float32, tag="sq_d")

                def rsqrt_via_reshape(src_sc, dst_row, drow):
                    # src_sc [1,S] sum-of-squares -> rn_d[dst_row] = rsqrt(v+eps)
                    nc.sync.dma_start(out=sq_d[drow:drow + 1, :], in_=src_sc)
                    rsh = npo.tile([26, 120], dt.float32, tag="rsh", bufs=2)
                    nc.sync.dma_start(
                        out=rsh, in_=sq_d[drow:drow + 1, :].rearrange(
                            "o (t p) -> (o t) p", p=120))
                    rsh2 = npo.tile([26, 120], dt.float32, tag="rsh2", bufs=2)
                    nc.scalar.activation(out=rsh2, in_=rsh, func=AF.Sqrt,
                                         bias=eps128[0:26], scale=1.0 / DIM)
                    nc.vector.reciprocal(rsh, rsh2)
                    nc.sync.dma_start(
                        out=rn_d[dst_row:dst_row + 1, :].rearrange(
                            "o (t p) -> (o t) p", p=120),
                        in_=rsh)

                # k norm
                r0 = npo.tile([1, S], dt.float32, tag="ssqr", bufs=3)
                nc.sync.dma_start(out=r0, in_=ssq_out[0:1, :])
                r1 = npo.tile([1, S], dt.float32, tag="ssqr", bufs=3)
                nc.sync.dma_start(out=r1, in_=ssq_out[1:2, :])
                nc.vector.tensor_add(sc1, r0, r1)
                rsqrt_via_reshape(sc1, 0, 0)
                nc.scalar.dma_start(out=bb,
                                    in_=rn_d[0:1, :].to_broadcast((128, S)))
                nc.vector.tensor_mul(kT[0][:, CACHED:], kT[0][:, CACHED:], bb)
                nc.vector.tensor_mul(kT[1][:, CACHED:], kT[1][:, CACHED:], bb)
                # q norm
                r2 = npo.tile([1, S], dt.float32, tag="ssqr", bufs=3)
                nc.sync.dma_start(out=r2, in_=ssq_out[2:3, :])
                r3 = npo.tile([1, S], dt.float32, tag="ssqr", bufs=3)
                nc.sync.dma_start(out=r3, in_=ssq_out[3:4, :])
                r4 = npo.tile([1, S], dt.float32, tag="ssqr", bufs=3)
                nc.sync.dma_start(out=r4, in_=ssq_out[4:5, :])
                nc.vector.tensor_add(sc1[:, 0:HALF], r2[:, 0:HALF], r3[:, 0:HALF])
                nc.vector.tensor_add(sc1[:, HALF:], r2[:, HALF:], r4[:, 0:HALF])
                rsqrt_via_reshape(sc1, 1, 1)
                sc3_d = rn_d  # rnq now in rn_d[1]
                nc.sync.dma_start(out=sc3, in_=rn_d[1:2, :])
                bb2 = npo.tile([128, S], dt.float32, tag="bb")
                nc.sync.dma_start(out=bb2,
                                  in_=rn_d[1:2, :].to_broadcast((128, S)))
                nc.vector.tensor_mul(qT0, qT0, bb2)
                # slot1 q norm select
                nc.vector.tensor_scalar(out=sc2[:, 0:HALF], in0=sc3[:, 0:HALF],
                                        scalar1=m_ap["rnsel"], scalar2=None,
                                        op0=OP.mult)
                nc.vector.tensor_scalar(out=sc2[:, HALF:], in0=sc3[:, HALF:],
                                        scalar1=m_ap["rnsel2"], scalar2=None,
                                        op0=OP.mult)
                nc.vector.tensor_add(sc1[:, 0:HALF], sc2[:, 0:HALF], sc2[:, HALF:])
                nc.sync.dma_start(out=rn_d[2:3, 0:HALF], in_=sc1[:, 0:HALF])
                bb3 = npo.tile([128, HALF], dt.float32, tag="bbh")
                nc.sync.dma_start(out=bb3,
                                  in_=rn_d[2:3, 0:HALF].to_broadcast((128, HALF)))
                nc.vector.tensor_mul(qT1, qT1, bb3)

            # ---------- phase 2: attention + o-proj ----------
            partial1 = dram.tile([S, DIM], dt.float16, tag="partial1")
            partial2 = dram.tile([HALF, DIM], dt.float16, tag="partial2")

            with (
                tc.tile_pool(name="stps", bufs=2, space="PSUM") as stps_p,
                tc.tile_pool(name="pvps", bufs=2, space="PSUM") as pvps_p,
                tc.tile_pool(name="rsps", bufs=2, space="PSUM") as rsps_p,
                tc.tile_pool(name="ptp2", bufs=4) as ptp2,
                tc.tile_pool(name="att_sb", bufs=3) as asb,
            ):
                def attention(slot, qchunks, qT_t, part_dram):
                    for qoff, qsz in qchunks:
                        pvps = pvps_p.tile([128, 512], dt.float32, tag="pv")
                        rsps = rsps_p.tile([128, 512], dt.float32, tag="rsop")
                        for g in EXP_GROUPS:
                            stp = stps_p.tile([128, 2, 512], dt.float32, tag="st")
                            for j, t in enumerate(g):
                                koff, ksz = KEY_TILES[t]
                                nc.tensor.matmul(
                                    stp[0:ksz, j, 0:qsz],
                                    kT[slot][:, koff:koff + ksz],
                                    qT_t[:, qoff:qoff + qsz],
                                    start=True, stop=True)
                            pt = ptp2.tile([128, 2, 512], dt.bfloat16, tag="pt")
                            gsz = KEY_TILES[g[0]][1] if len(g) == 1 else 128
                            nc.scalar.activation(
                                out=pt[0:gsz, 0:len(g), 0:qsz],
                                in_=stp[0:gsz, 0:len(g), 0:qsz],
                                func=AF.Exp, bias=0.0, scale=SCALE)
                            for j, t in enumerate(g):
                                koff, ksz = KEY_TILES[t]
                                nc.tensor.matmul(
                                    pvps[:, 0:qsz], v_sb[slot][0:ksz, t, :],
                                    pt[0:ksz, j, 0:qsz],
                                    start=(t == 0), stop=(t == N_KT - 1))
                                nc.tensor.matmul(
                                    rsps[0:1, 0:qsz], ones_b[0:ksz, :],
                                    pt[0:ksz, j, 0:qsz],
                                    start=(t == 0), stop=(t == N_KT - 1))
                        rc = asb.tile([1, 512], dt.float32, tag="rc")
                        nc.vector.tensor_copy(rc[:, 0:qsz], rsps[0:1, 0:qsz])
                        rc_d = dram.tile([1, 512], dt.float32, tag="rc_d")
                        nc.sync.dma_start(out=rc_d[:, 0:qsz], in_=rc[:, 0:qsz])
                        rsb = asb.tile([128, 512], dt.float32, tag="rsb")
                        nc.sync.dma_start(
                            out=rsb[:, 0:qsz],
                            in_=rc_d[0:1, 0:qsz].to_broadcast((128, qsz)))
                        rcb = asb.tile([128, 512], dt.float32, tag="rcb")
                        nc.vector.reciprocal(rcb[:, 0:qsz], rsb[:, 0:qsz])
                        oT = asb.tile([128, 512], dt.float32r, tag="oT")
                        nc.vector.tensor_mul(oT[:, 0:qsz], pvps[:, 0:qsz],
                                             rcb[:, 0:qsz])
                        for stoff in range(0, qsz, 128):
                            stsz = min(128, qsz - stoff)
                            for ec in range(3):
                                ops = rsps_p.tile([128, 512], dt.float32, tag="rsop")
                                nc.tensor.matmul(
                                    ops[0:stsz, :], oT[:, stoff:stoff + stsz],
                                    wo_r[slot][:, ec * 512:(ec + 1) * 512],
                                    start=True, stop=True)
                                ev = asb.tile([128, 512], dt.float16, tag="ev")
                                nc.any.tensor_copy(ev[0:stsz, :], ops[0:stsz, :])
                                nc.sync.dma_start(
                                    out=part_dram[qoff + stoff:qoff + stoff + stsz,
                                                  ec * 512:(ec + 1) * 512],
                                    in_=ev[0:stsz, :])

                rsA = dram.tile([OUT_ROWS, DIM], dt.float16, tag="rsA")
                rsB = dram.tile([OUT_ROWS, DIM], dt.float16, tag="rsB")
                attention(0, Q_CHUNKS0, qT0, partial1)
                # RS for slot0 partials overlaps slot1 attention
                nc.gpsimd.collective_compute(
                    "ReduceScatter", OP.add,
                    replica_groups=[list(range(N_CORES))],
                    ins=[partial1.opt()], outs=[rsA.opt()],
                )
                attention(1, Q_CHUNKS1, qT1, partial2)
                nc.gpsimd.collective_compute(
                    "ReduceScatter", OP.add,
                    replica_groups=[[0, 1, 2, 3], [4, 5, 6, 7]],
                    ins=[partial2.opt()], outs=[rsB.opt()],
                )
            with tc.tile_pool(name="fin", bufs=2) as fin:
                bo_sb = fin.tile([128, DIM], dt.float32, tag="bo")
                nc.sync.dma_start(out=bo_sb, in_=bo128)
                for roff in range(0, OUT_ROWS, 128):
                    rsz = min(128, OUT_ROWS - roff)
                    ta = fin.tile([128, DIM], dt.float16, tag="fa")
                    tb = fin.tile([128, DIM], dt.float16, tag="fb")
                    tf = fin.tile([128, DIM], dt.float32, tag="ff")
                    nc.sync.dma_start(out=ta[0:rsz, :], in_=rsA[roff:roff + rsz, :])
                    nc.sync.dma_start(out=tb[0:rsz, :], in_=rsB[roff:roff + rsz, :])
                    nc.vector.tensor_add(tf[0:rsz, :], ta[0:rsz, :], tb[0:rsz, :])
                    nc.vector.tensor_add(tf[0:rsz, :], tf[0:rsz, :], bo_sb[0:rsz, :])
                    nc.sync.dma_start(out=out_ap[roff:roff + rsz, :],
                                      in_=tf[0:rsz, :])

    nc.compile()
    _BUILD_CACHE["nc"] = nc
    return nc


PERM = np.concatenate([np.arange(0, 128, 2), np.arange(1, 128, 2)])
PERM_SW = np.concatenate([PERM[64:], PERM[:64]])


def _host_prep(inputs):
    x = np.asarray(inputs["x"])[0]  # [S, DIM]
    theta = np.asarray(inputs["freqs_theta"])
    cache_k = np.asarray(inputs["cache_k"])[0]  # [L, 12, 128]
    cache_v = np.asarray(inputs["cache_v"])[0]
    wq, wk, wv, wo = (np.asarray(inputs[n]) for n in ("wq", "wk", "wv", "wo"))
    bq, bk, bv, bo = (np.asarray(inputs[n]) for n in ("bq", "bk", "bv", "bo"))
    gq, gk = np.asarray(inputs["gq"]), np.asarray(inputs["gk"])
    f, h, w = int(inputs["grid_f"]), int(inputs["grid_h"]), int(inputs["grid_w"])
    current_start = int(inputs["current_start"])
    global_end = int(inputs["global_end_index"])
    local_end_in = int(inputs["local_end_index"])

    frame_seqlen = h * w
    start_frame = current_start // frame_seqlen
    current_end = current_start + S
    local_end = local_end_in + current_end - global_end
    local_start = local_end - S
    win_start = max(0, local_end - MAX_ATTN)
    assert local_start - win_start == CACHED, (win_start, local_start)

    # rope angle table [S, 64]
    c = HD // 2
    ct_ = c - 2 * (c // 3)  # 22
    ch_ = c // 3  # 21
    ang = np.concatenate([
        np.broadcast_to(theta[start_frame:start_frame + f, :ct_][:, None, None, :],
                        (f, h, w, ct_)),
        np.broadcast_to(theta[:h, ct_:ct_ + ch_][None, :, None, :], (f, h, w, ch_)),
        np.broadcast_to(theta[:w, ct_ + ch_:ct_ + 2 * ch_][None, None, :, :],
                        (f, h, w, ch_)),
    ], axis=-1).reshape(S, c)
    cosT = np.cos(ang).T.astype(np.float32)  # [64, S]
    sinT = np.sin(ang).T.astype(np.float32)
    cosD = np.ascontiguousarray(np.concatenate([cosT, cosT], 0))  # [128, S]
    sinD = np.ascontiguousarray(np.concatenate([-sinT, sinT], 0))

    xTf = np.ascontiguousarray(x.T, np.float32)  # [DIM, S]

    def wslice(wm, head, perm):
        block = wm[head * HD:(head + 1) * HD, :][perm, :]  # [128, DIM]
        return np.ascontiguousarray(block.T, np.float32)  # [DIM, 128]

    def col(vec, head, perm):
        return np.ascontiguousarray(
            vec[head * HD:(head + 1) * HD][perm][:, None], np.float32)

    in_maps = []
    for cidx in range(N_CORES):
        h0 = cidx
        h1 = 8 + (cidx % 4)
        half = cidx // 4
        hsl = slice(half * HALF, (half + 1) * HALF)
        kc = []
        vc = []
        for hh in (h0, h1):
            arr = cache_k[win_start:local_start, hh, :]  # [CACHED, 128]
            kc.append(np.ascontiguousarray(arr.T[PERM], np.float32))
            vc.append(np.ascontiguousarray(
                cache_v[win_start:local_start, hh, :]).astype(ml_dtypes.bfloat16))
        iden = np.arange(128)
        m = {
            "xT": xTf,
            "xu": np.ascontiguousarray(xTf[:, hsl]),
            "wq0": wslice(wq, h0, PERM), "wq1": wslice(wq, h1, PERM),
            "wk0": wslice(wk, h0, PERM), "wk1": wslice(wk, h1, PERM),
            "wv0": wslice(wv, h0, iden), "wv1": wslice(wv, h1, iden),
            "bq0": col(bq, h0, PERM), "bq1": col(bq, h1, PERM),
            "bk0": col(bk, h0, PERM), "bk1": col(bk, h1, PERM),
            "bv0": col(bv, h0, iden), "bv1": col(bv, h1, iden),
            "gq0": col(gq, h0, PERM), "gq0s": col(gq, h0, PERM_SW),
            "gq1": col(gq, h1, PERM), "gq1s": col(gq, h1, PERM_SW),
            "gk0": col(gk, h0, PERM), "gk0s": col(gk, h0, PERM_SW),
            "gk1": col(gk, h1, PERM), "gk1s": col(gk, h1, PERM_SW),
            "cosq": cosD, "sinq": sinD,
            "cosq1": np.ascontiguousarray(cosD[:, hsl]),
            "sinq1": np.ascontiguousarray(sinD[:, hsl]),
            "kc0": kc[0], "kc1": kc[1], "vc0": vc[0], "vc1": vc[1],
            "wo0": np.ascontiguousarray(wo[:, h0 * HD:(h0 + 1) * HD].T, np.float32),
            "wo1": np.ascontiguousarray(wo[:, h1 * HD:(h1 + 1) * HD].T, np.float32),
            "bo128": np.broadcast_to(bo[None, :], (128, DIM)).astype(np.float32).copy(),
            "mk1": np.full((1, 1), 1.0 if cidx < 4 else 0.0, np.float32),
            "mq3": np.full((1, 1), 1.0 if half == 0 else 0.0, np.float32),
            "mq4": np.full((1, 1), 1.0 if half == 1 else 0.0, np.float32),
            "rnsel": np.full((1, 1), 1.0 if half == 0 else 0.0, np.float32),
            "rnsel2": np.full((1, 1), 1.0 if half == 1 else 0.0, np.float32),
        }
        in_maps.append(m)
    return in_maps


def kernel(**inputs):
    from concourse.bass_utils import run_bass_kernel_spmd

    nc = _build()
    in_maps = _host_prep(inputs)
    res = run_bass_kernel_spmd(nc, in_maps, list(range(N_CORES)))
    out = np.concatenate([res.results[i]["out"] for i in range(N_CORES)], axis=0)
    return out[None].astype(np.float32)


# revision 14
# speedup vs baseline: 1.0566x; 1.0566x over previous
"""CausalWanSelfAttention TRN2 kernel: 8-core SPMD via bass/tile.

Sharding: head-tensor-parallel with query-split for the 4 "extra" heads.
 - core c slot0: head c (0..7), all S=3120 queries, full KV.
 - core c slot1: head 8+(c%4), query-half (c//4), full KV (KV dup x2).
RMS-norm coupling across heads handled by a tiny ssq AllReduce.
Output projection partials combined by ReduceScatter (global 8-core for
slot0, two 4-core groups for slot1 halves), concatenated on host.

All matmuls f32r (QK, projections, o-proj) except softmax*V which is bf16.
q/k feature order is permuted (evens-then-odds within each head) so RoPE
becomes partition-contiguous half-block ops; scores are invariant.
"""

import math
import sys

import numpy as np

sys.path.insert(0, "/opt/trn_rl_repo")

import ml_dtypes  # noqa: E402

NUM_HEADS = 12
DIM = 1536
HD = 128
S = 3120
HALF = 1560
CACHED = 2512
KEYS = CACHED + S  # 5632
MAX_ATTN = 5632
EPS = 1e-6
CT = DIM // 128  # 12
N_CORES = 8
SCALE = 1.0 / math.sqrt(HD)
OUT_ROWS = S // N_CORES  # 390

# key tiles: 19 full cached, 1x80 cached, 24 full new, 1x48 new
KEY_TILES = (
    [(i * 128, 128) for i in range(19)]
    + [(2432, 80)]
    + [(CACHED + i * 128, 128) for i in range(24)]
    + [(CACHED + 3072, 48)]
)
N_KT = len(KEY_TILES)  # 45

EXP_GROUPS = (
    [(i, i + 1) for i in range(0, 18, 2)]
    + [(18,), (19,)]
    + [(i, i + 1) for i in range(20, 44, 2)]
    + [(44,)]
)

S_CHUNKS = [(i * 256, 256) for i in range(12)] + [(3072, 48)]
H_CHUNKS = [(i * 256, 256) for i in range(6)] + [(1536, 24)]
Q_CHUNKS0 = [(i * 512, 512) for i in range(6)] + [(3072, 48)]
Q_CHUNKS1 = [(i * 512, 512) for i in range(3)] + [(1536, 24)]

_BUILD_CACHE = {}


def _build():
    if "nc" in _BUILD_CACHE:
        return _BUILD_CACHE["nc"]

    import concourse.mybir as mybir
    import concourse.tile as tile
    from concourse import bacc
    from concourse.masks import make_identity

    dt = mybir.dt
    AF = mybir.ActivationFunctionType
    OP = mybir.AluOpType

    nc = bacc.Bacc("TRN2", num_devices=N_CORES, debug=False)

    def din(name, shape, dtype=dt.float32):
        return nc.dram_tensor(name, shape, dtype, kind="ExternalInput").ap()

    xT = din("xT", [DIM, S])
    xu = din("xu", [DIM, HALF])
    w_in = {n: din(n, [DIM, 128]) for n in ("wq0", "wq1", "wk0", "wk1", "wv0", "wv1")}
    b_in = {n: din(n, [128, 1]) for n in ("bq0", "bq1", "bk0", "bk1", "bv0", "bv1")}
    g_in = {
        n: din(n, [128, 1])
        for n in ("gq0", "gq0s", "gq1", "gq1s", "gk0", "gk0s", "gk1", "gk1s")
    }
    cosq = din("cosq", [128, S])
    sinq = din("sinq", [128, S])
    cosq1 = din("cosq1", [128, HALF])
    sinq1 = din("sinq1", [128, HALF])
    kc_in = [din("kc0", [128, CACHED]), din("kc1", [128, CACHED])]
    vc_in = [din("vc0", [CACHED, 128], dt.bfloat16), din("vc1", [CACHED, 128], dt.bfloat16)]
    wo_in = [din("wo0", [128, DIM]), din("wo1", [128, DIM])]
    bo128 = din("bo128", [128, DIM])
    masks = {n: din(n, [1, 1]) for n in ("mk1", "mq3", "mq4", "rnsel", "rnsel2")}
    out_ap = nc.dram_tensor("out", [OUT_ROWS, DIM], dt.float32,
                            kind="ExternalOutput").ap()

    with tile.TileContext(nc) as tc:
        with (
            tc.tile_pool(name="persist", bufs=1) as persist,
            tc.tile_pool(name="dram", bufs=1, space="DRAM") as dram,
        ):
            # ---------- persistent tiles ----------
            kT = [persist.tile([128, KEYS], dt.float32r, tag=f"kT{i}", name=f"kT{i}")
                  for i in range(2)]
            qT0 = persist.tile([128, S], dt.float32r, tag="qT0")
            qT1 = persist.tile([128, HALF], dt.float32r, tag="qT1")
            v_sb = [persist.tile([128, N_KT, 128], dt.bfloat16, tag=f"v{i}", name=f"v{i}")
                    for i in range(2)]
            wo_r = [persist.tile([128, DIM], dt.float32r, tag=f"wo{i}", name=f"wo{i}")
                    for i in range(2)]
            ones_r = persist.tile([128, 1], dt.float32r, tag="ones_r")
            ones_b = persist.tile([128, 1], dt.bfloat16, tag="ones_b")
            ident = persist.tile([128, 128], dt.float32, tag="ident")
            m_ap = {}
            for n in masks:
                t = persist.tile([1, 1], dt.float32, tag=f"m_{n}", name=f"m_{n}")
                nc.sync.dma_start(out=t, in_=masks[n])
                m_ap[n] = t

            make_identity(nc, ident)
            tmp1 = persist.tile([128, 1], dt.float32, tag="tmp1")
            nc.vector.memset(tmp1, 1.0)
            nc.vector.tensor_copy(ones_r, tmp1)
            nc.vector.memset(ones_b, 1.0)

            # cached V -> v tiles (early; overlaps with proj compute)
            for i in range(2):
                nc.sync.dma_start(
                    out=v_sb[i][:, 0:19, :],
                    in_=vc_in[i][0:2432, :].rearrange("(t p) d -> p t d", p=128),
                )
                nc.sync.dma_start(out=v_sb[i][0:80, 19, :], in_=vc_in[i][2432:2512, :])

            ssq_in = dram.tile([8, S], dt.float32, tag="ssq_in")
            ssq_out = dram.tile([8, S], dt.float32, tag="ssq_out")

            # ---------- phase 1: projections + rope ----------
            with (
                tc.tile_pool(name="bgp", bufs=1) as bgp,
                tc.tile_pool(name="xstage", bufs=2) as xs,
                tc.tile_pool(name="xr", bufs=2) as xrp,
                tc.tile_pool(name="praw", bufs=3) as prp,
                tc.tile_pool(name="ptmp", bufs=2) as ptp,
                tc.tile_pool(name="ssqs", bufs=3) as ssqsb,
                tc.tile_pool(name="p1ps", bufs=4, space="PSUM") as p1ps,
                tc.tile_pool(name="ssqps", bufs=2, space="PSUM") as ssqps,
                tc.tile_pool(name="tps", bufs=2, space="PSUM") as tps,
            ):
                b_ap = {}
                for n, src in list(b_in.items()) + list(g_in.items()):
                    t = bgp.tile([128, 1], dt.float32, tag=f"b_{n}", name=f"b_{n}")
                    nc.sync.dma_start(out=t, in_=src)
                    b_ap[n] = t

                def proj_chunk(xsrc, soff, ssz, specs, ctab_d, stab_d):
                    raw = xs.tile([128, CT, 256], dt.float32, tag="x")
                    nc.sync.dma_start(
                        out=raw[:, :, 0:ssz],
                        in_=xsrc.rearrange("(ct p) s -> p ct s", p=128)[
                            :, :, soff:soff + ssz],
                    )
                    ctab = ptp.tile([128, 256], dt.float32, tag="ctab")
                    stab = ptp.tile([128, 256], dt.float32, tag="stab")
                    nc.sync.dma_start(out=ctab[:, 0:ssz],
                                      in_=ctab_d[:, soff:soff + ssz])
                    nc.sync.dma_start(out=stab[:, 0:ssz],
                                      in_=stab_d[:, soff:soff + ssz])
                    xr = xrp.tile([128, CT, 256], dt.float32r, tag="xr")
                    nc.scalar.activation(out=xr[:, :, 0:ssz], in_=raw[:, :, 0:ssz],
                                         func=AF.Copy, bias=0.0, scale=1.0)
                    for spec in specs:
                        kind, wname, bname = spec[:3]
                        ps = p1ps.tile([128, 256], dt.float32, tag="projps")
                        for ct in range(CT):
                            nc.tensor.matmul(ps[:, 0:ssz], w_r[wname][:, ct],
                                             xr[:, ct, 0:ssz],
                                             start=(ct == 0), stop=(ct == CT - 1))
                        pr = prp.tile([128, 256], dt.float32, tag="praw")
                        nc.vector.tensor_scalar(
                            out=pr[:, 0:ssz], in0=ps[:, 0:ssz],
                            scalar1=b_ap[bname], scalar2=None, op0=OP.add)
                        if kind == "v":
                            vi = spec[3]
                            for bo_ in range(0, ssz, 128):
                                bsz = min(128, ssz - bo_)
                                ptile = tps.tile([128, 128], dt.float32, tag="tp")
                                nc.tensor.transpose(
                                    ptile[0:bsz, :], pr[:, bo_:bo_ + bsz], ident)
                                ti = 20 + (soff + bo_) // 128
                                nc.vector.tensor_copy(
                                    v_sb[vi][0:bsz, ti, :], ptile[0:bsz, :])
                        else:
                            rows, gname, gsname, dest = spec[3:]
                            sq = ptp.tile([128, 256], dt.float32r, tag="sq")
                            nc.vector.tensor_mul(sq[:, 0:ssz], pr[:, 0:ssz],
                                                 pr[:, 0:ssz])
                            sps = ssqps.tile([1, 256], dt.float32, tag="ssqps")
                            nc.tensor.matmul(sps[0:1, 0:ssz], ones_r, sq[:, 0:ssz],
                                             start=True, stop=True)
                            for row, mask in rows:
                                srow = ssqsb.tile([1, 256], dt.float32, tag="ssq")
                                nc.vector.tensor_scalar(
                                    out=srow[0:1, 0:ssz],
                                    in0=sps[0:1, 0:ssz],
                                    scalar1=(m_ap[mask] if mask else 1.0),
                                    scalar2=None, op0=OP.mult)
                                nc.sync.dma_start(
                                    out=ssq_in[row:row + 1, soff:soff + ssz],
                                    in_=srow[0:1, 0:ssz])
                            # rope: dest = (pr*g) . cos + (swap(pr)*gs) . sin
                            sh = ptp.tile([128, 256], dt.float32, tag="sh")
                            nc.sync.dma_start(out=sh[0:64, 0:ssz],
                                              in_=pr[64:128, 0:ssz])
                            nc.sync.dma_start(out=sh[64:128, 0:ssz],
                                              in_=pr[0:64, 0:ssz])
                            tc_ = ptp.tile([128, 256], dt.float32, tag="tcos")
                            nc.vector.scalar_tensor_tensor(
                                out=tc_[:, 0:ssz], in0=pr[:, 0:ssz],
                                scalar=b_ap[gname],
                                in1=ctab[:, 0:ssz],
                                op0=OP.mult, op1=OP.mult)
                            ts_ = ptp.tile([128, 256], dt.float32, tag="tsin")
                            nc.vector.scalar_tensor_tensor(
                                out=ts_[:, 0:ssz], in0=sh[:, 0:ssz],
                                scalar=b_ap[gsname],
                                in1=stab[:, 0:ssz],
                                op0=OP.mult, op1=OP.mult)
                            nc.vector.tensor_add(dest[:, soff:soff + ssz],
                                                 tc_[:, 0:ssz], ts_[:, 0:ssz])

                # --- main S loop (slot0 q/k/v + slot1 k/v) ---
                with (
                    tc.tile_pool(name="wts", bufs=1) as wpool,
                    tc.tile_pool(name="wstage", bufs=1) as ws,
                ):
                    w_r = {}
                    for n in ("wq0", "wk0", "wv0", "wk1", "wv1"):
                        rawt = ws.tile([128, CT, 128], dt.float32, tag="wstage")
                        nc.sync.dma_start(
                            out=rawt, in_=w_in[n].rearrange("(ct p) f -> p ct f", p=128))
                        wr = wpool.tile([128, CT, 128], dt.float32r, tag=f"w_{n}")
                        nc.vector.tensor_copy(wr, rawt)
                        w_r[n] = wr

                    for soff, ssz in S_CHUNKS:
                        proj_chunk(
                            xT, soff, ssz,
                            [
                                ("qk", "wq0", "bq0", [(2, None)], "gq0", "gq0s", qT0),
                                ("qk", "wk0", "bk0", [(0, None)], "gk0", "gk0s",
                                 kT[0][:, CACHED:]),
                                ("v", "wv0", "bv0", 0),
                                ("qk", "wk1", "bk1", [(1, "mk1")], "gk1", "gk1s",
                                 kT[1][:, CACHED:]),
                                ("v", "wv1", "bv1", 1),
                            ],
                            cosq, sinq,
                        )

                # --- slot1 q loop ---
                with (
                    tc.tile_pool(name="wts1", bufs=1) as wpool1,
                    tc.tile_pool(name="wstage1", bufs=1) as ws1,
                ):
                    rawt = ws1.tile([128, CT, 128], dt.float32, tag="wstage")
                    nc.sync.dma_start(
                        out=rawt, in_=w_in["wq1"].rearrange("(ct p) f -> p ct f", p=128))
                    wq1r = wpool1.tile([128, CT, 128], dt.float32r, tag="w_wq1")
                    nc.vector.tensor_copy(wq1r, rawt)
                    w_r["wq1"] = wq1r
                    for soff, ssz in H_CHUNKS:
                        proj_chunk(
                            xu, soff, ssz,
                            [("qk", "wq1", "bq1", [(3, "mq3"), (4, "mq4")],
                              "gq1", "gq1s", qT1)],
                            cosq1, sinq1,
                        )

            # ---------- ssq AllReduce (issued before V so it overlaps) ----------
            nc.gpsimd.collective_compute(
                "AllReduce", OP.add,
                replica_groups=[list(range(N_CORES))],
                ins=[ssq_in.opt()], outs=[ssq_out.opt()],
            )

            # ---------- cached K load (round to f32r) ----------
            with tc.tile_pool(name="kcstage", bufs=2) as kcs:
                for i in range(2):
                    t = kcs.tile([128, CACHED], dt.float32, tag="kc")
                    nc.sync.dma_start(out=t, in_=kc_in[i])
                    nc.vector.tensor_copy(kT[i][:, 0:CACHED], t)
                # round wo while we're at it
                for i in range(2):
                    t2 = kcs.tile([128, DIM], dt.float32, tag="wos")
                    nc.sync.dma_start(out=t2, in_=wo_in[i])
                    nc.vector.tensor_copy(wo_r[i], t2)
            with tc.tile_pool(name="normp", bufs=1) as npo:
                eps_t = npo.tile([1, 1], dt.float32, tag="eps")
                nc.vector.memset(eps_t, EPS)
                rn_d = dram.tile([3, S], dt.float32, tag="rn_d")
                sc1 = npo.tile([1, S], dt.float32, tag="sc1")
                sc2 = npo.tile([1, S], dt.float32, tag="sc2")
                sc3 = npo.tile([1, S], dt.float32, tag="sc3")
                bb = npo.tile([128, S], dt.float32, tag="bb")
                eps128 = npo.tile([128, 1], dt.float32, tag="eps128")
                nc.vector.memset(eps128, EPS)
                sq_d = dram.tile([2, S], dt.float32, tag="sq_d")

                def rsqrt_via_reshape(src_sc, dst_row, drow):
                    # src_sc [1,S] sum-of-squares -> rn_d[dst_row] = rsqrt(v+eps)
                    nc.sync.dma_start(out=sq_d[drow:drow + 1, :], in_=src_sc)
                    rsh = npo.tile([26, 120], dt.float32, tag="rsh", bufs=2)
                    nc.sync.dma_start(
                        out=rsh, in_=sq_d[drow:drow + 1, :].rearrange(
                            "o (t p) -> (o t) p", p=120))
                    rsh2 = npo.tile([26, 120], dt.float32, tag="rsh2", bufs=2)
                    nc.scalar.activation(out=rsh2, in_=rsh, func=AF.Sqrt,
                                         bias=eps128[0:26], scale=1.0 / DIM)
                    nc.vector.reciprocal(rsh, rsh2)
                    nc.sync.dma_start(
                        out=rn_d[dst_row:dst_row + 1, :].rearrange(
                            "o (t p) -> (o t) p", p=120),
                        in_=rsh)

                # k norm
                r0 = npo.tile([1, S], dt.float32, tag="ssqr", bufs=3)
                nc.sync.dma_start(out=r0, in_=ssq_out[0:1, :])
                r1 = npo.tile([1, S], dt.float32, tag="ssqr", bufs=3)
                nc.sync.dma_start(out=r1, in_=ssq_out[1:2, :])
                nc.vector.tensor_add(sc1, r0, r1)
                rsqrt_via_reshape(sc1, 0, 0)
                nc.scalar.dma_start(out=bb,
                                    in_=rn_d[0:1, :].to_broadcast((128, S)))
                nc.vector.tensor_mul(kT[0][:, CACHED:], kT[0][:, CACHED:], bb)
                nc.vector.tensor_mul(kT[1][:, CACHED:], kT[1][:, CACHED:], bb)
                # q norm
                r2 = npo.tile([1, S], dt.float32, tag="ssqr", bufs=3)
                nc.sync.dma_start(out=r2, in_=ssq_out[2:3, :])
                r3 = npo.tile([1, S], dt.float32, tag="ssqr", bufs=3)
                nc.sync.dma_start(out=r3, in_=ssq_out[3:4, :])
                r4 = npo.tile([1, S], dt.float32, tag="ssqr", bufs=3)
                nc.sync.dma_start(out=r4, in_=ssq_out[4:5, :])
                nc.vector.tensor_add(sc1[:, 0:HALF], r2[:, 0:HALF], r3[:, 0:HALF])
                nc.vector.tensor_add(sc1[:, HALF:], r2[:, HALF:], r4[:, 0:HALF])
                rsqrt_via_reshape(sc1, 1, 1)
                sc3_d = rn_d  # rnq now in rn_d[1]
                nc.sync.dma_start(out=sc3, in_=rn_d[1:2, :])
                bb2 = npo.tile([128, S], dt.float32, tag="bb")
                nc.sync.dma_start(out=bb2,
                                  in_=rn_d[1:2, :].to_broadcast((128, S)))
                nc.vector.tensor_mul(qT0, qT0, bb2)
                # slot1 q norm select
                nc.vector.tensor_scalar(out=sc2[:, 0:HALF], in0=sc3[:, 0:HALF],
                                        scalar1=m_ap["rnsel"], scalar2=None,
                                        op0=OP.mult)
                nc.vector.tensor_scalar(out=sc2[:, HALF:], in0=sc3[:, HALF:],
                                        scalar1=m_ap["rnsel2"], scalar2=None,
                                        op0=OP.mult)
                nc.vector.tensor_add(sc1[:, 0:HALF], sc2[:, 0:HALF], sc2[:, HALF:])
                nc.sync.dma_start(out=rn_d[2:3, 0:HALF], in_=sc1[:, 0:HALF])
                bb3 = npo.tile([128, HALF], dt.float32, tag="bbh")
                nc.sync.dma_start(out=bb3,
                                  in_=rn_d[2:3, 0:HALF].to_broadcast((128, HALF)))
                nc.vector.tensor_mul(qT1, qT1, bb3)

            # ---------- phase 2: attention + o-proj ----------
            partial1 = dram.tile([S, DIM], dt.float16, tag="partial1")
            partial2 = dram.tile([HALF, DIM], dt.float16, tag="partial2")

            with (
                tc.tile_pool(name="stps", bufs=2, space="PSUM") as stps_p,
                tc.tile_pool(name="pvps", bufs=2, space="PSUM") as pvps_p,
                tc.tile_pool(name="rsps", bufs=2, space="PSUM") as rsps_p,
                tc.tile_pool(name="ptp2", bufs=4) as ptp2,
                tc.tile_pool(name="att_sb", bufs=3) as asb,
            ):
                def attention(slot, qchunks, qT_t, part_dram):
                    for qoff, qsz in qchunks:
                        pvps = pvps_p.tile([128, 512], dt.float32, tag="pv")
                        rsps = rsps_p.tile([128, 512], dt.float32, tag="rsop")
                        for g in EXP_GROUPS:
                            stp = stps_p.tile([128, 2, 512], dt.float32, tag="st")
                            for j, t in enumerate(g):
                                koff, ksz = KEY_TILES[t]
                                nc.tensor.matmul(
                                    stp[0:ksz, j, 0:qsz],
                                    kT[slot][:, koff:koff + ksz],
                                    qT_t[:, qoff:qoff + qsz],
                                    start=True, stop=True)
                            pt = ptp2.tile([128, 2, 512], dt.bfloat16, tag="pt")
                            gsz = KEY_TILES[g[0]][1] if len(g) == 1 else 128
                            nc.scalar.activation(
                                out=pt[0:gsz, 0:len(g), 0:qsz],
                                in_=stp[0:gsz, 0:len(g), 0:qsz],
                                func=AF.Exp, bias=0.0, scale=SCALE)
                            for j, t in enumerate(g):
                                koff, ksz = KEY_TILES[t]
                                nc.tensor.matmul(
                                    pvps[:, 0:qsz], v_sb[slot][0:ksz, t, :],
                                    pt[0:ksz, j, 0:qsz],
                                    start=(t == 0), stop=(t == N_KT - 1))
                                nc.tensor.matmul(
                                    rsps[0:1, 0:qsz], ones_b[0:ksz, :],
                                    pt[0:ksz, j, 0:qsz],
                                    start=(t == 0), stop=(t == N_KT - 1))
                        rc = asb.tile([1, 512], dt.float32, tag="rc")
                        nc.vector.tensor_copy(rc[:, 0:qsz], rsps[0:1, 0:qsz])
                        rc_d = dram.tile([1, 512], dt.float32, tag="rc_d")
                        nc.sync.dma_start(out=rc_d[:, 0:qsz], in_=rc[:, 0:qsz])
                        rsb = asb.tile([128, 512], dt.float32, tag="rsb")
                        nc.sync.dma_start(
                            out=rsb[:, 0:qsz],
                            in_=rc_d[0:1, 0:qsz].to_broadcast((128, qsz)))
                        rcb = asb.tile([128, 512], dt.float32, tag="rcb")
                        nc.vector.reciprocal(rcb[:, 0:qsz], rsb[:, 0:qsz])
                        oT = asb.tile([128, 512], dt.float32r, tag="oT")
                        nc.vector.tensor_mul(oT[:, 0:qsz], pvps[:, 0:qsz],
                                             rcb[:, 0:qsz])
                        for stoff in range(0, qsz, 128):
                            stsz = min(128, qsz - stoff)
                            for ec in range(3):
                                ops = rsps_p.tile([128, 512], dt.float32, tag="rsop")
                                nc.tensor.matmul(
                                    ops[0:stsz, :], oT[:, stoff:stoff + stsz],
                                    wo_r[slot][:, ec * 512:(ec + 1) * 512],
                                    start=True, stop=True)
                                ev = asb.tile([128, 512], dt.float16, tag="ev")
                                nc.any.tensor_copy(ev[0:stsz, :], ops[0:stsz, :])
                                nc.sync.dma_start(
                                    out=part_dram[qoff + stoff:qoff + stoff + stsz,
                                                  ec * 512:(ec + 1) * 512],
                                    in_=ev[0:stsz, :])

                rsA = dram.tile([OUT_ROWS, DIM], dt.float16, tag="rsA")
                rsB = dram.tile([OUT_ROWS, DIM], dt.float16, tag="rsB")
                attention(0, Q_CHUNKS0, qT0, partial1)
                # RS for slot0 partials overlaps slot1 attention
                nc.gpsimd.collective_compute(
                    "ReduceScatter", OP.add,
                    replica_groups=[list(range(N_CORES))],
                    ins=[partial1.opt()], outs=[rsA.opt()],
                )
                attention(1, Q_CHUNKS1, qT1, partial2)
                nc.gpsimd.collective_compute(
                    "ReduceScatter", OP.add,
                    replica_groups=[[0, 1, 2, 3], [4, 5, 6, 7]],
                    ins=[partial2.opt()], outs=[rsB.opt()],
                )
            with tc.tile_pool(name="fin", bufs=2) as fin:
                bo_sb = fin.tile([128, DIM], dt.float32, tag="bo")
                nc.sync.dma_start(out=bo_sb, in_=bo128)
                for roff in range(0, OUT_ROWS, 128):
                    rsz = min(128, OUT_ROWS - roff)
                    ta = fin.tile([128, DIM], dt.float16, tag="fa")
                    tb = fin.tile([128, DIM], dt.float16, tag="fb")
                    tf = fin.tile([128, DIM], dt.float32, tag="ff")
                    nc.sync.dma_start(out=ta[0:rsz, :], in_=rsA[roff:roff + rsz, :])
                    nc.sync.dma_start(out=tb[0:rsz, :], in_=rsB[roff:roff + rsz, :])
                    nc.vector.tensor_add(tf[0:rsz, :], ta[0:rsz, :], tb[0:rsz, :])
                    nc.vector.tensor_add(tf[0:rsz, :], tf[0:rsz, :], bo_sb[0:rsz, :])
                    nc.sync.dma_start(out=out_ap[roff:roff + rsz, :],
                                      in_=tf[0:rsz, :])

    nc.compile()
    _BUILD_CACHE["nc"] = nc
    return nc


PERM = np.concatenate([np.arange(0, 128, 2), np.arange(1, 128, 2)])
PERM_SW = np.concatenate([PERM[64:], PERM[:64]])


def _host_prep(inputs):
    x = np.asarray(inputs["x"])[0]  # [S, DIM]
    theta = np.asarray(inputs["freqs_theta"])
    cache_k = np.asarray(inputs["cache_k"])[0]  # [L, 12, 128]
    cache_v = np.asarray(inputs["cache_v"])[0]
    wq, wk, wv, wo = (np.asarray(inputs[n]) for n in ("wq", "wk", "wv", "wo"))
    bq, bk, bv, bo = (np.asarray(inputs[n]) for n in ("bq", "bk", "bv", "bo"))
    gq, gk = np.asarray(inputs["gq"]), np.asarray(inputs["gk"])
    f, h, w = int(inputs["grid_f"]), int(inputs["grid_h"]), int(inputs["grid_w"])
    current_start = int(inputs["current_start"])
    global_end = int(inputs["global_end_index"])
    local_end_in = int(inputs["local_end_index"])

    frame_seqlen = h * w
    start_frame = current_start // frame_seqlen
    current_end = current_start + S
    local_end = local_end_in + current_end - global_end
    local_start = local_end - S
    win_start = max(0, local_end - MAX_ATTN)
    assert local_start - win_start == CACHED, (win_start, local_start)

    # rope angle table [S, 64]
    c = HD // 2
    ct_ = c - 2 * (c // 3)  # 22
    ch_ = c // 3  # 21
    ang = np.concatenate([
        np.broadcast_to(theta[start_frame:start_frame + f, :ct_][:, None, None, :],
                        (f, h, w, ct_)),
        np.broadcast_to(theta[:h, ct_:ct_ + ch_][None, :, None, :], (f, h, w, ch_)),
        np.broadcast_to(theta[:w, ct_ + ch_:ct_ + 2 * ch_][None, None, :, :],
                        (f, h, w, ch_)),
    ], axis=-1).reshape(S, c)
    cosT = np.cos(ang).T.astype(np.float32)  # [64, S]
    sinT = np.sin(ang).T.astype(np.float32)
    cosD = np.ascontiguousarray(np.concatenate([cosT, cosT], 0))  # [128, S]
    sinD = np.ascontiguousarray(np.concatenate([-sinT, sinT], 0))

    xTf = np.ascontiguousarray(x.T, np.float32)  # [DIM, S]

    def wslice(wm, head, perm):
        block = wm[head * HD:(head + 1) * HD, :][perm, :]  # [128, DIM]
        return np.ascontiguousarray(block.T, np.float32)  # [DIM, 128]

    def col(vec, head, perm):
        return np.ascontiguousarray(
            vec[head * HD:(head + 1) * HD][perm][:, None], np.float32)

    in_maps = []
    for cidx in range(N_CORES):
        h0 = cidx
        h1 = 8 + (cidx % 4)
        half = cidx // 4
        hsl = slice(half * HALF, (half + 1) * HALF)
        kc = []
        vc = []
        for hh in (h0, h1):
            arr = cache_k[win_start:local_start, hh, :]  # [CACHED, 128]
            kc.append(np.ascontiguousarray(arr.T[PERM], np.float32))
            vc.append(np.ascontiguousarray(
                cache_v[win_start:local_start, hh, :]).astype(ml_dtypes.bfloat16))
        iden = np.arange(128)
        m = {
            "xT": xTf,
            "xu": np.ascontiguousarray(xTf[:, hsl]),
            "wq0": wslice(wq, h0, PERM), "wq1": wslice(wq, h1, PERM),
            "wk0": wslice(wk, h0, PERM), "wk1": wslice(wk, h1, PERM),
            "wv0": wslice(wv, h0, iden), "wv1": wslice(wv, h1, iden),
            "bq0": col(bq, h0, PERM), "bq1": col(bq, h1, PERM),
            "bk0": col(bk, h0, PERM), "bk1": col(bk, h1, PERM),
            "bv0": col(bv, h0, iden), "bv1": col(bv, h1, iden),
            "gq0": col(gq, h0, PERM), "gq0s": col(gq, h0, PERM_SW),
            "gq1": col(gq, h1, PERM), "gq1s": col(gq, h1, PERM_SW),
            "gk0": col(gk, h0, PERM), "gk0s": col(gk, h0, PERM_SW),
            "gk1": col(gk, h1, PERM), "gk1s": col(gk, h1, PERM_SW),
            "cosq": cosD, "sinq": sinD,
            "cosq1": np.ascontiguousarray(cosD[:, hsl]),
            "sinq1": np.ascontiguousarray(sinD[:, hsl]),
            "kc0": kc[0], "kc1": kc[1], "vc0": vc[0], "vc1": vc[1],
            "wo0": np.ascontiguousarray(wo[:, h0 * HD:(h0 + 1) * HD].T, np.float32),
            "wo1": np.ascontiguousarray(wo[:, h1 * HD:(h1 + 1) * HD].T, np.float32),
            "bo128": np.broadcast_to(bo[None, :], (128, DIM)).astype(np.float32).copy(),
            "mk1": np.full((1, 1), 1.0 if cidx < 4 else 0.0, np.float32),
            "mq3": np.full((1, 1), 1.0 if half == 0 else 0.0, np.float32),
            "mq4": np.full((1, 1), 1.0 if half == 1 else 0.0, np.float32),
            "rnsel": np.full((1, 1), 1.0 if half == 0 else 0.0, np.float32),
            "rnsel2": np.full((1, 1), 1.0 if half == 1 else 0.0, np.float32),
        }
        in_maps.append(m)
    return in_maps


def kernel(**inputs):
    from concourse.bass_utils import run_bass_kernel_spmd

    nc = _build()
    in_maps = _host_prep(inputs)
    res = run_bass_kernel_spmd(nc, in_maps, list(range(N_CORES)))
    out = np.concatenate([res.results[i]["out"] for i in range(N_CORES)], axis=0)
    return out[None].astype(np.float32)


# revision 15
# speedup vs baseline: 1.0576x; 1.0009x over previous
"""CausalWanSelfAttention TRN2 kernel: 8-core SPMD via bass/tile.

Sharding: head-tensor-parallel with query-split for the 4 "extra" heads.
 - core c slot0: head c (0..7), all S=3120 queries, full KV.
 - core c slot1: head 8+(c%4), query-half (c//4), full KV (KV dup x2).
RMS-norm coupling across heads handled by a tiny ssq AllReduce.
Output projection partials combined by ReduceScatter (global 8-core for
slot0, two 4-core groups for slot1 halves), concatenated on host.

All matmuls f32r (QK, projections, o-proj) except softmax*V which is bf16.
q/k feature order is permuted (evens-then-odds within each head) so RoPE
becomes partition-contiguous half-block ops; scores are invariant.
"""

import math
import sys

import numpy as np

sys.path.insert(0, "/opt/trn_rl_repo")

import ml_dtypes  # noqa: E402

NUM_HEADS = 12
DIM = 1536
HD = 128
S = 3120
HALF = 1560
CACHED = 2512
KEYS = CACHED + S  # 5632
MAX_ATTN = 5632
EPS = 1e-6
CT = DIM // 128  # 12
N_CORES = 8
SCALE = 1.0 / math.sqrt(HD)
OUT_ROWS = S // N_CORES  # 390

# key tiles: 19 full cached, 1x80 cached, 24 full new, 1x48 new
KEY_TILES = (
    [(i * 128, 128) for i in range(19)]
    + [(2432, 80)]
    + [(CACHED + i * 128, 128) for i in range(24)]
    + [(CACHED + 3072, 48)]
)
N_KT = len(KEY_TILES)  # 45

EXP_GROUPS = (
    [(i, i + 1) for i in range(0, 18, 2)]
    + [(18,), (19,)]
    + [(i, i + 1) for i in range(20, 44, 2)]
    + [(44,)]
)

S_CHUNKS = [(i * 256, 256) for i in range(12)] + [(3072, 48)]
H_CHUNKS = [(i * 256, 256) for i in range(6)] + [(1536, 24)]
Q_CHUNKS0 = [(i * 512, 512) for i in range(6)] + [(3072, 48)]
Q_CHUNKS1 = [(i * 512, 512) for i in range(3)] + [(1536, 24)]

_BUILD_CACHE = {}


def _build():
    if "nc" in _BUILD_CACHE:
        return _BUILD_CACHE["nc"]

    import concourse.mybir as mybir
    import concourse.tile as tile
    from concourse import bacc
    from concourse.masks import make_identity

    dt = mybir.dt
    AF = mybir.ActivationFunctionType
    OP = mybir.AluOpType

    nc = bacc.Bacc("TRN2", num_devices=N_CORES, debug=False)

    def din(name, shape, dtype=dt.float32):
        return nc.dram_tensor(name, shape, dtype, kind="ExternalInput").ap()

    xT = din("xT", [DIM, S])
    xu = din("xu", [DIM, HALF])
    w_in = {n: din(n, [DIM, 128]) for n in ("wq0", "wq1", "wk0", "wk1", "wv0", "wv1")}
    b_in = {n: din(n, [128, 1]) for n in ("bq0", "bq1", "bk0", "bk1", "bv0", "bv1")}
    g_in = {
        n: din(n, [128, 1])
        for n in ("gq0", "gq0s", "gq1", "gq1s", "gk0", "gk0s", "gk1", "gk1s")
    }
    cosq = din("cosq", [128, S])
    sinq = din("sinq", [128, S])
    cosq1 = din("cosq1", [128, HALF])
    sinq1 = din("sinq1", [128, HALF])
    kc_in = [din("kc0", [128, CACHED]), din("kc1", [128, CACHED])]
    vc_in = [din("vc0", [CACHED, 128], dt.bfloat16), din("vc1", [CACHED, 128], dt.bfloat16)]
    wo_in = [din("wo0", [128, DIM]), din("wo1", [128, DIM])]
    bo128 = din("bo128", [128, DIM])
    masks = {n: din(n, [1, 1]) for n in ("mk1", "mq3", "mq4", "rnsel", "rnsel2")}
    out_ap = nc.dram_tensor("out", [OUT_ROWS, DIM], dt.float32,
                            kind="ExternalOutput").ap()

    with tile.TileContext(nc) as tc:
        with (
            tc.tile_pool(name="persist", bufs=1) as persist,
            tc.tile_pool(name="dram", bufs=1, space="DRAM") as dram,
        ):
            # ---------- persistent tiles ----------
            kT = [persist.tile([128, KEYS], dt.float32r, tag=f"kT{i}", name=f"kT{i}")
                  for i in range(2)]
            qT0 = persist.tile([128, S], dt.float32r, tag="qT0")
            qT1 = persist.tile([128, HALF], dt.float32r, tag="qT1")
            v_sb = [persist.tile([128, N_KT, 128], dt.bfloat16, tag=f"v{i}", name=f"v{i}")
                    for i in range(2)]
            wo_r = [persist.tile([128, DIM], dt.float32r, tag=f"wo{i}", name=f"wo{i}")
                    for i in range(2)]
            ones_r = persist.tile([128, 1], dt.float32r, tag="ones_r")
            ones_b = persist.tile([128, 1], dt.bfloat16, tag="ones_b")
            ident = persist.tile([128, 128], dt.float32, tag="ident")
            m_ap = {}
            for n in masks:
                t = persist.tile([1, 1], dt.float32, tag=f"m_{n}", name=f"m_{n}")
                nc.sync.dma_start(out=t, in_=masks[n])
                m_ap[n] = t

            make_identity(nc, ident)
            tmp1 = persist.tile([128, 1], dt.float32, tag="tmp1")
            nc.vector.memset(tmp1, 1.0)
            nc.vector.tensor_copy(ones_r, tmp1)
            nc.vector.memset(ones_b, 1.0)

            # cached V -> v tiles (early; overlaps with proj compute)
            for i in range(2):
                nc.sync.dma_start(
                    out=v_sb[i][:, 0:19, :],
                    in_=vc_in[i][0:2432, :].rearrange("(t p) d -> p t d", p=128),
                )
                nc.sync.dma_start(out=v_sb[i][0:80, 19, :], in_=vc_in[i][2432:2512, :])

            ssq_in = dram.tile([8, S], dt.float32, tag="ssq_in")
            ssq_out = dram.tile([8, S], dt.float32, tag="ssq_out")

            # ---------- phase 1: projections + rope ----------
            with (
                tc.tile_pool(name="bgp", bufs=1) as bgp,
                tc.tile_pool(name="xstage", bufs=2) as xs,
                tc.tile_pool(name="xr", bufs=2) as xrp,
                tc.tile_pool(name="praw", bufs=3) as prp,
                tc.tile_pool(name="ptmp", bufs=2) as ptp,
                tc.tile_pool(name="ssqs", bufs=3) as ssqsb,
                tc.tile_pool(name="p1ps", bufs=4, space="PSUM") as p1ps,
                tc.tile_pool(name="ssqps", bufs=2, space="PSUM") as ssqps,
                tc.tile_pool(name="tps", bufs=2, space="PSUM") as tps,
            ):
                b_ap = {}
                for n, src in list(b_in.items()) + list(g_in.items()):
                    t = bgp.tile([128, 1], dt.float32, tag=f"b_{n}", name=f"b_{n}")
                    nc.sync.dma_start(out=t, in_=src)
                    b_ap[n] = t

                def proj_chunk(xsrc, soff, ssz, specs, ctab_d, stab_d):
                    raw = xs.tile([128, CT, 256], dt.float32, tag="x")
                    nc.sync.dma_start(
                        out=raw[:, :, 0:ssz],
                        in_=xsrc.rearrange("(ct p) s -> p ct s", p=128)[
                            :, :, soff:soff + ssz],
                    )
                    ctab = ptp.tile([128, 256], dt.float32, tag="ctab")
                    stab = ptp.tile([128, 256], dt.float32, tag="stab")
                    nc.sync.dma_start(out=ctab[:, 0:ssz],
                                      in_=ctab_d[:, soff:soff + ssz])
                    nc.sync.dma_start(out=stab[:, 0:ssz],
                                      in_=stab_d[:, soff:soff + ssz])
                    xr = xrp.tile([128, CT, 256], dt.float32r, tag="xr")
                    nc.scalar.activation(out=xr[:, :, 0:ssz], in_=raw[:, :, 0:ssz],
                                         func=AF.Copy, bias=0.0, scale=1.0)
                    for spec in specs:
                        kind, wname, bname = spec[:3]
                        ps = p1ps.tile([128, 256], dt.float32, tag="projps")
                        for ct in range(CT):
                            nc.tensor.matmul(ps[:, 0:ssz], w_r[wname][:, ct],
                                             xr[:, ct, 0:ssz],
                                             start=(ct == 0), stop=(ct == CT - 1))
                        pr = prp.tile([128, 256], dt.float32, tag="praw")
                        nc.vector.tensor_scalar(
                            out=pr[:, 0:ssz], in0=ps[:, 0:ssz],
                            scalar1=b_ap[bname], scalar2=None, op0=OP.add)
                        if kind == "v":
                            vi = spec[3]
                            for bo_ in range(0, ssz, 128):
                                bsz = min(128, ssz - bo_)
                                ptile = tps.tile([128, 128], dt.float32, tag="tp")
                                nc.tensor.transpose(
                                    ptile[0:bsz, :], pr[:, bo_:bo_ + bsz], ident)
                                ti = 20 + (soff + bo_) // 128
                                nc.vector.tensor_copy(
                                    v_sb[vi][0:bsz, ti, :], ptile[0:bsz, :])
                        else:
                            rows, gname, gsname, dest = spec[3:]
                            sq = ptp.tile([128, 256], dt.float32r, tag="sq")
                            nc.vector.tensor_mul(sq[:, 0:ssz], pr[:, 0:ssz],
                                                 pr[:, 0:ssz])
                            sps = ssqps.tile([1, 256], dt.float32, tag="ssqps")
                            nc.tensor.matmul(sps[0:1, 0:ssz], ones_r, sq[:, 0:ssz],
                                             start=True, stop=True)
                            for row, mask in rows:
                                srow = ssqsb.tile([1, 256], dt.float32, tag="ssq")
                                nc.vector.tensor_scalar(
                                    out=srow[0:1, 0:ssz],
                                    in0=sps[0:1, 0:ssz],
                                    scalar1=(m_ap[mask] if mask else 1.0),
                                    scalar2=None, op0=OP.mult)
                                nc.sync.dma_start(
                                    out=ssq_in[row:row + 1, soff:soff + ssz],
                                    in_=srow[0:1, 0:ssz])
                            # rope: dest = (pr*g) . cos + (swap(pr)*gs) . sin
                            sh = ptp.tile([128, 256], dt.float32, tag="sh")
                            nc.sync.dma_start(out=sh[0:64, 0:ssz],
                                              in_=pr[64:128, 0:ssz])
                            nc.sync.dma_start(out=sh[64:128, 0:ssz],
                                              in_=pr[0:64, 0:ssz])
                            tc_ = ptp.tile([128, 256], dt.float32, tag="tcos")
                            nc.vector.scalar_tensor_tensor(
                                out=tc_[:, 0:ssz], in0=pr[:, 0:ssz],
                                scalar=b_ap[gname],
                                in1=ctab[:, 0:ssz],
                                op0=OP.mult, op1=OP.mult)
                            ts_ = ptp.tile([128, 256], dt.float32, tag="tsin")
                            nc.vector.scalar_tensor_tensor(
                                out=ts_[:, 0:ssz], in0=sh[:, 0:ssz],
                                scalar=b_ap[gsname],
                                in1=stab[:, 0:ssz],
                                op0=OP.mult, op1=OP.mult)
                            nc.vector.tensor_add(dest[:, soff:soff + ssz],
                                                 tc_[:, 0:ssz], ts_[:, 0:ssz])

                # --- main S loop (slot0 q/k/v + slot1 k/v) ---
                with (
                    tc.tile_pool(name="wts", bufs=1) as wpool,
                    tc.tile_pool(name="wstage", bufs=1) as ws,
                ):
                    w_r = {}
                    for n in ("wq0", "wk0", "wv0", "wk1", "wv1"):
                        rawt = ws.tile([128, CT, 128], dt.float32, tag="wstage")
                        nc.sync.dma_start(
                            out=rawt, in_=w_in[n].rearrange("(ct p) f -> p ct f", p=128))
                        wr = wpool.tile([128, CT, 128], dt.float32r, tag=f"w_{n}")
                        nc.vector.tensor_copy(wr, rawt)
                        w_r[n] = wr

                    for soff, ssz in S_CHUNKS:
                        proj_chunk(
                            xT, soff, ssz,
                            [
                                ("qk", "wq0", "bq0", [(2, None)], "gq0", "gq0s", qT0),
                                ("qk", "wk0", "bk0", [(0, None)], "gk0", "gk0s",
                                 kT[0][:, CACHED:]),
                                ("v", "wv0", "bv0", 0),
                                ("qk", "wk1", "bk1", [(1, "mk1")], "gk1", "gk1s",
                                 kT[1][:, CACHED:]),
                                ("v", "wv1", "bv1", 1),
                            ],
                            cosq, sinq,
                        )

                # --- slot1 q loop ---
                with (
                    tc.tile_pool(name="wts1", bufs=1) as wpool1,
                    tc.tile_pool(name="wstage1", bufs=1) as ws1,
                ):
                    rawt = ws1.tile([128, CT, 128], dt.float32, tag="wstage")
                    nc.sync.dma_start(
                        out=rawt, in_=w_in["wq1"].rearrange("(ct p) f -> p ct f", p=128))
                    wq1r = wpool1.tile([128, CT, 128], dt.float32r, tag="w_wq1")
                    nc.vector.tensor_copy(wq1r, rawt)
                    w_r["wq1"] = wq1r
                    for soff, ssz in H_CHUNKS:
                        proj_chunk(
                            xu, soff, ssz,
                            [("qk", "wq1", "bq1", [(3, "mq3"), (4, "mq4")],
                              "gq1", "gq1s", qT1)],
                            cosq1, sinq1,
                        )

            # ---------- ssq AllReduce (issued before V so it overlaps) ----------
            nc.gpsimd.collective_compute(
                "AllReduce", OP.add,
                replica_groups=[list(range(N_CORES))],
                ins=[ssq_in.opt()], outs=[ssq_out.opt()],
            )

            # ---------- cached K load (round to f32r) ----------
            with tc.tile_pool(name="kcstage", bufs=2) as kcs:
                for i in range(2):
                    t = kcs.tile([128, CACHED], dt.float32, tag="kc")
                    nc.sync.dma_start(out=t, in_=kc_in[i])
                    nc.vector.tensor_copy(kT[i][:, 0:CACHED], t)
                # round wo while we're at it
                for i in range(2):
                    t2 = kcs.tile([128, DIM], dt.float32, tag="wos")
                    nc.sync.dma_start(out=t2, in_=wo_in[i])
                    nc.vector.tensor_copy(wo_r[i], t2)
            with tc.tile_pool(name="normp", bufs=1) as npo:
                eps_t = npo.tile([1, 1], dt.float32, tag="eps")
                nc.vector.memset(eps_t, EPS)
                rn_d = dram.tile([3, S], dt.float32, tag="rn_d")
                sc1 = npo.tile([1, S], dt.float32, tag="sc1")
                sc2 = npo.tile([1, S], dt.float32, tag="sc2")
                sc3 = npo.tile([1, S], dt.float32, tag="sc3")
                bb = npo.tile([128, S], dt.float32, tag="bb")
                eps128 = npo.tile([128, 1], dt.float32, tag="eps128")
                nc.vector.memset(eps128, EPS)
                sq_d = dram.tile([2, S], dt.float32, tag="sq_d")

                def rsqrt_via_reshape(src_sc, dst_row, drow):
                    # src_sc [1,S] sum-of-squares -> rn_d[dst_row] = rsqrt(v+eps)
                    nc.sync.dma_start(out=sq_d[drow:drow + 1, :], in_=src_sc)
                    rsh = npo.tile([26, 120], dt.float32, tag="rsh", bufs=2)
                    nc.sync.dma_start(
                        out=rsh, in_=sq_d[drow:drow + 1, :].rearrange(
                            "o (t p) -> (o t) p", p=120))
                    rsh2 = npo.tile([26, 120], dt.float32, tag="rsh2", bufs=2)
                    nc.scalar.activation(out=rsh2, in_=rsh, func=AF.Sqrt,
                                         bias=eps128[0:26], scale=1.0 / DIM)
                    nc.vector.reciprocal(rsh, rsh2)
                    nc.sync.dma_start(
                        out=rn_d[dst_row:dst_row + 1, :].rearrange(
                            "o (t p) -> (o t) p", p=120),
                        in_=rsh)

                # k norm
                r0 = npo.tile([1, S], dt.float32, tag="ssqr", bufs=3)
                nc.sync.dma_start(out=r0, in_=ssq_out[0:1, :])
                r1 = npo.tile([1, S], dt.float32, tag="ssqr", bufs=3)
                nc.sync.dma_start(out=r1, in_=ssq_out[1:2, :])
                nc.vector.tensor_add(sc1, r0, r1)
                rsqrt_via_reshape(sc1, 0, 0)
                nc.scalar.dma_start(out=bb,
                                    in_=rn_d[0:1, :].to_broadcast((128, S)))
                nc.vector.tensor_mul(kT[0][:, CACHED:], kT[0][:, CACHED:], bb)
                nc.vector.tensor_mul(kT[1][:, CACHED:], kT[1][:, CACHED:], bb)
                # q norm
                r2 = npo.tile([1, S], dt.float32, tag="ssqr", bufs=3)
                nc.sync.dma_start(out=r2, in_=ssq_out[2:3, :])
                r3 = npo.tile([1, S], dt.float32, tag="ssqr", bufs=3)
                nc.sync.dma_start(out=r3, in_=ssq_out[3:4, :])
                r4 = npo.tile([1, S], dt.float32, tag="ssqr", bufs=3)
                nc.sync.dma_start(out=r4, in_=ssq_out[4:5, :])
                nc.vector.tensor_add(sc1[:, 0:HALF], r2[:, 0:HALF], r3[:, 0:HALF])
                nc.vector.tensor_add(sc1[:, HALF:], r2[:, HALF:], r4[:, 0:HALF])
                rsqrt_via_reshape(sc1, 1, 1)
                sc3_d = rn_d  # rnq now in rn_d[1]
                nc.sync.dma_start(out=sc3, in_=rn_d[1:2, :])
                bb2 = npo.tile([128, S], dt.float32, tag="bb")
                nc.sync.dma_start(out=bb2,
                                  in_=rn_d[1:2, :].to_broadcast((128, S)))
                nc.vector.tensor_mul(qT0, qT0, bb2)
                # slot1 q norm select
                nc.vector.tensor_scalar(out=sc2[:, 0:HALF], in0=sc3[:, 0:HALF],
                                        scalar1=m_ap["rnsel"], scalar2=None,
                                        op0=OP.mult)
                nc.vector.tensor_scalar(out=sc2[:, HALF:], in0=sc3[:, HALF:],
                                        scalar1=m_ap["rnsel2"], scalar2=None,
                                        op0=OP.mult)
                nc.vector.tensor_add(sc1[:, 0:HALF], sc2[:, 0:HALF], sc2[:, HALF:])
                nc.sync.dma_start(out=rn_d[2:3, 0:HALF], in_=sc1[:, 0:HALF])
                bb3 = npo.tile([128, HALF], dt.float32, tag="bbh")
                nc.sync.dma_start(out=bb3,
                                  in_=rn_d[2:3, 0:HALF].to_broadcast((128, HALF)))
                nc.vector.tensor_mul(qT1, qT1, bb3)

            # ---------- phase 2: attention + o-proj ----------
            partial1 = dram.tile([S, DIM], dt.float16, tag="partial1")
            partial2 = dram.tile([HALF, DIM], dt.float16, tag="partial2")

            with (
                tc.tile_pool(name="stps", bufs=3, space="PSUM") as stps_p,
                tc.tile_pool(name="pvps", bufs=1, space="PSUM") as pvps_p,
                tc.tile_pool(name="rsps", bufs=1, space="PSUM") as rsps_p,
                tc.tile_pool(name="ptp2", bufs=4) as ptp2,
                tc.tile_pool(name="att_sb", bufs=3) as asb,
            ):
                def attention(slot, qchunks, qT_t, part_dram):
                    for qoff, qsz in qchunks:
                        pvps = pvps_p.tile([128, 512], dt.float32, tag="pv")
                        rsps = rsps_p.tile([128, 512], dt.float32, tag="rsop")
                        for g in EXP_GROUPS:
                            stp = stps_p.tile([128, 2, 512], dt.float32, tag="st")
                            for j, t in enumerate(g):
                                koff, ksz = KEY_TILES[t]
                                nc.tensor.matmul(
                                    stp[0:ksz, j, 0:qsz],
                                    kT[slot][:, koff:koff + ksz],
                                    qT_t[:, qoff:qoff + qsz],
                                    start=True, stop=True)
                            pt = ptp2.tile([128, 2, 512], dt.bfloat16, tag="pt")
                            gsz = KEY_TILES[g[0]][1] if len(g) == 1 else 128
                            nc.scalar.activation(
                                out=pt[0:gsz, 0:len(g), 0:qsz],
                                in_=stp[0:gsz, 0:len(g), 0:qsz],
                                func=AF.Exp, bias=0.0, scale=SCALE)
                            for j, t in enumerate(g):
                                koff, ksz = KEY_TILES[t]
                                nc.tensor.matmul(
                                    pvps[:, 0:qsz], v_sb[slot][0:ksz, t, :],
                                    pt[0:ksz, j, 0:qsz],
                                    start=(t == 0), stop=(t == N_KT - 1))
                                nc.tensor.matmul(
                                    rsps[0:1, 0:qsz], ones_b[0:ksz, :],
                                    pt[0:ksz, j, 0:qsz],
                                    start=(t == 0), stop=(t == N_KT - 1))
                        rc = asb.tile([1, 512], dt.float32, tag="rc")
                        nc.vector.tensor_copy(rc[:, 0:qsz], rsps[0:1, 0:qsz])
                        rc_d = dram.tile([1, 512], dt.float32, tag="rc_d")
                        nc.sync.dma_start(out=rc_d[:, 0:qsz], in_=rc[:, 0:qsz])
                        rsb = asb.tile([128, 512], dt.float32, tag="rsb")
                        nc.sync.dma_start(
                            out=rsb[:, 0:qsz],
                            in_=rc_d[0:1, 0:qsz].to_broadcast((128, qsz)))
                        rcb = asb.tile([128, 512], dt.float32, tag="rcb")
                        nc.vector.reciprocal(rcb[:, 0:qsz], rsb[:, 0:qsz])
                        oT = asb.tile([128, 512], dt.float32r, tag="oT")
                        nc.vector.tensor_mul(oT[:, 0:qsz], pvps[:, 0:qsz],
                                             rcb[:, 0:qsz])
                        for stoff in range(0, qsz, 128):
                            stsz = min(128, qsz - stoff)
                            for ec in range(3):
                                ops = rsps_p.tile([128, 512], dt.float32, tag="rsop")
                                nc.tensor.matmul(
                                    ops[0:stsz, :], oT[:, stoff:stoff + stsz],
                                    wo_r[slot][:, ec * 512:(ec + 1) * 512],
                                    start=True, stop=True)
                                ev = asb.tile([128, 512], dt.float16, tag="ev")
                                nc.any.tensor_copy(ev[0:stsz, :], ops[0:stsz, :])
                                nc.sync.dma_start(
                                    out=part_dram[qoff + stoff:qoff + stoff + stsz,
                                                  ec * 512:(ec + 1) * 512],
                                    in_=ev[0:stsz, :])

                rsA = dram.tile([OUT_ROWS, DIM], dt.float16, tag="rsA")
                rsB = dram.tile([OUT_ROWS, DIM], dt.float16, tag="rsB")
                attention(0, Q_CHUNKS0, qT0, partial1)
                # RS for slot0 partials overlaps slot1 attention
                nc.gpsimd.collective_compute(
                    "ReduceScatter", OP.add,
                    replica_groups=[list(range(N_CORES))],
                    ins=[partial1.opt()], outs=[rsA.opt()],
                )
                attention(1, Q_CHUNKS1, qT1, partial2)
                nc.gpsimd.collective_compute(
                    "ReduceScatter", OP.add,
                    replica_groups=[[0, 1, 2, 3], [4, 5, 6, 7]],
                    ins=[partial2.opt()], outs=[rsB.opt()],
                )
            with tc.tile_pool(name="fin", bufs=2) as fin:
                bo_sb = fin.tile([128, DIM], dt.float32, tag="bo")
                nc.sync.dma_start(out=bo_sb, in_=bo128)
                for roff in range(0, OUT_ROWS, 128):
                    rsz = min(128, OUT_ROWS - roff)
                    ta = fin.tile([128, DIM], dt.float16, tag="fa")
                    tb = fin.tile([128, DIM], dt.float16, tag="fb")
                    tf = fin.tile([128, DIM], dt.float32, tag="ff")
                    nc.sync.dma_start(out=ta[0:rsz, :], in_=rsA[roff:roff + rsz, :])
                    nc.sync.dma_start(out=tb[0:rsz, :], in_=rsB[roff:roff + rsz, :])
                    nc.vector.tensor_add(tf[0:rsz, :], ta[0:rsz, :], tb[0:rsz, :])
                    nc.vector.tensor_add(tf[0:rsz, :], tf[0:rsz, :], bo_sb[0:rsz, :])
                    nc.sync.dma_start(out=out_ap[roff:roff + rsz, :],
                                      in_=tf[0:rsz, :])

    nc.compile()
    _BUILD_CACHE["nc"] = nc
    return nc


PERM = np.concatenate([np.arange(0, 128, 2), np.arange(1, 128, 2)])
PERM_SW = np.concatenate([PERM[64:], PERM[:64]])


def _host_prep(inputs):
    x = np.asarray(inputs["x"])[0]  # [S, DIM]
    theta = np.asarray(inputs["freqs_theta"])
    cache_k = np.asarray(inputs["cache_k"])[0]  # [L, 12, 128]
    cache_v = np.asarray(inputs["cache_v"])[0]
    wq, wk, wv, wo = (np.asarray(inputs[n]) for n in ("wq", "wk", "wv", "wo"))
    bq, bk, bv, bo = (np.asarray(inputs[n]) for n in ("bq", "bk", "bv", "bo"))
    gq, gk = np.asarray(inputs["gq"]), np.asarray(inputs["gk"])
    f, h, w = int(inputs["grid_f"]), int(inputs["grid_h"]), int(inputs["grid_w"])
    current_start = int(inputs["current_start"])
    global_end = int(inputs["global_end_index"])
    local_end_in = int(inputs["local_end_index"])

    frame_seqlen = h * w
    start_frame = current_start // frame_seqlen
    current_end = current_start + S
    local_end = local_end_in + current_end - global_end
    local_start = local_end - S
    win_start = max(0, local_end - MAX_ATTN)
    assert local_start - win_start == CACHED, (win_start, local_start)

    # rope angle table [S, 64]
    c = HD // 2
    ct_ = c - 2 * (c // 3)  # 22
    ch_ = c // 3  # 21
    ang = np.concatenate([
        np.broadcast_to(theta[start_frame:start_frame + f, :ct_][:, None, None, :],
                        (f, h, w, ct_)),
        np.broadcast_to(theta[:h, ct_:ct_ + ch_][None, :, None, :], (f, h, w, ch_)),
        np.broadcast_to(theta[:w, ct_ + ch_:ct_ + 2 * ch_][None, None, :, :],
                        (f, h, w, ch_)),
    ], axis=-1).reshape(S, c)
    cosT = np.cos(ang).T.astype(np.float32)  # [64, S]
    sinT = np.sin(ang).T.astype(np.float32)
    cosD = np.ascontiguousarray(np.concatenate([cosT, cosT], 0))  # [128, S]
    sinD = np.ascontiguousarray(np.concatenate([-sinT, sinT], 0))

    xTf = np.ascontiguousarray(x.T, np.float32)  # [DIM, S]

    def wslice(wm, head, perm):
        block = wm[head * HD:(head + 1) * HD, :][perm, :]  # [128, DIM]
        return np.ascontiguousarray(block.T, np.float32)  # [DIM, 128]

    def col(vec, head, perm):
        return np.ascontiguousarray(
            vec[head * HD:(head + 1) * HD][perm][:, None], np.float32)

    in_maps = []
    for cidx in range(N_CORES):
        h0 = cidx
        h1 = 8 + (cidx % 4)
        half = cidx // 4
        hsl = slice(half * HALF, (half + 1) * HALF)
        kc = []
        vc = []
        for hh in (h0, h1):
            arr = cache_k[win_start:local_start, hh, :]  # [CACHED, 128]
            kc.append(np.ascontiguousarray(arr.T[PERM], np.float32))
            vc.append(np.ascontiguousarray(
                cache_v[win_start:local_start, hh, :]).astype(ml_dtypes.bfloat16))
        iden = np.arange(128)
        m = {
            "xT": xTf,
            "xu": np.ascontiguousarray(xTf[:, hsl]),
            "wq0": wslice(wq, h0, PERM), "wq1": wslice(wq, h1, PERM),
            "wk0": wslice(wk, h0, PERM), "wk1": wslice(wk, h1, PERM),
            "wv0": wslice(wv, h0, iden), "wv1": wslice(wv, h1, iden),
            "bq0": col(bq, h0, PERM), "bq1": col(bq, h1, PERM),
            "bk0": col(bk, h0, PERM), "bk1": col(bk, h1, PERM),
            "bv0": col(bv, h0, iden), "bv1": col(bv, h1, iden),
            "gq0": col(gq, h0, PERM), "gq0s": col(gq, h0, PERM_SW),
            "gq1": col(gq, h1, PERM), "gq1s": col(gq, h1, PERM_SW),
            "gk0": col(gk, h0, PERM), "gk0s": col(gk, h0, PERM_SW),
            "gk1": col(gk, h1, PERM), "gk1s": col(gk, h1, PERM_SW),
            "cosq": cosD, "sinq": sinD,
            "cosq1": np.ascontiguousarray(cosD[:, hsl]),
            "sinq1": np.ascontiguousarray(sinD[:, hsl]),
            "kc0": kc[0], "kc1": kc[1], "vc0": vc[0], "vc1": vc[1],
            "wo0": np.ascontiguousarray(wo[:, h0 * HD:(h0 + 1) * HD].T, np.float32),
            "wo1": np.ascontiguousarray(wo[:, h1 * HD:(h1 + 1) * HD].T, np.float32),
            "bo128": np.broadcast_to(bo[None, :], (128, DIM)).astype(np.float32).copy(),
            "mk1": np.full((1, 1), 1.0 if cidx < 4 else 0.0, np.float32),
            "mq3": np.full((1, 1), 1.0 if half == 0 else 0.0, np.float32),
            "mq4": np.full((1, 1), 1.0 if half == 1 else 0.0, np.float32),
            "rnsel": np.full((1, 1), 1.0 if half == 0 else 0.0, np.float32),
            "rnsel2": np.full((1, 1), 1.0 if half == 1 else 0.0, np.float32),
        }
        in_maps.append(m)
    return in_maps


def kernel(**inputs):
    from concourse.bass_utils import run_bass_kernel_spmd

    nc = _build()
    in_maps = _host_prep(inputs)
    res = run_bass_kernel_spmd(nc, in_maps, list(range(N_CORES)))
    out = np.concatenate([res.results[i]["out"] for i in range(N_CORES)], axis=0)
    return out[None].astype(np.float32)


# revision 16
# speedup vs baseline: 1.0839x; 1.0249x over previous
"""CausalWanSelfAttention TRN2 kernel: 8-core SPMD via bass/tile.

Sharding: head-tensor-parallel with query-split for the 4 "extra" heads.
 - core c slot0: head c (0..7), all S=3120 queries, full KV.
 - core c slot1: head 8+(c%4), query-half (c//4), full KV (KV dup x2).
RMS-norm coupling across heads handled by a tiny ssq AllReduce.
Output projection partials combined by ReduceScatter (global 8-core for
slot0, two 4-core groups for slot1 halves), concatenated on host.

All matmuls f32r (QK, projections, o-proj) except softmax*V which is bf16.
q/k feature order is permuted (evens-then-odds within each head) so RoPE
becomes partition-contiguous half-block ops; scores are invariant.
"""

import math
import sys

import numpy as np

sys.path.insert(0, "/opt/trn_rl_repo")

import ml_dtypes  # noqa: E402

NUM_HEADS = 12
DIM = 1536
HD = 128
S = 3120
HALF = 1560
CACHED = 2512
KEYS = CACHED + S  # 5632
MAX_ATTN = 5632
EPS = 1e-6
CT = DIM // 128  # 12
N_CORES = 8
SCALE = 1.0 / math.sqrt(HD)
OUT_ROWS = S // N_CORES  # 390

# key tiles: 19 full cached, 1x80 cached, 24 full new, 1x48 new
KEY_TILES = (
    [(i * 128, 128) for i in range(19)]
    + [(2432, 80)]
    + [(CACHED + i * 128, 128) for i in range(24)]
    + [(CACHED + 3072, 48)]
)
N_KT = len(KEY_TILES)  # 45

EXP_GROUPS = (
    [(i, i + 1) for i in range(0, 18, 2)]
    + [(18,), (19,)]
    + [(i, i + 1) for i in range(20, 44, 2)]
    + [(44,)]
)

S_CHUNKS = [(i * 256, 256) for i in range(12)] + [(3072, 48)]
H_CHUNKS = [(i * 256, 256) for i in range(6)] + [(1536, 24)]
Q_CHUNKS0 = [(i * 512, 512) for i in range(6)] + [(3072, 48)]
Q_CHUNKS1 = [(i * 512, 512) for i in range(3)] + [(1536, 24)]

_BUILD_CACHE = {}


def _build():
    if "nc" in _BUILD_CACHE:
        return _BUILD_CACHE["nc"]

    import concourse.mybir as mybir
    import concourse.tile as tile
    from concourse import bacc
    from concourse.masks import make_identity

    dt = mybir.dt
    AF = mybir.ActivationFunctionType
    OP = mybir.AluOpType

    nc = bacc.Bacc("TRN2", num_devices=N_CORES, debug=False)

    def din(name, shape, dtype=dt.float32):
        return nc.dram_tensor(name, shape, dtype, kind="ExternalInput").ap()

    xT = din("xT", [DIM, S], dt.float32r)
    xu = din("xu", [DIM, HALF], dt.float32r)
    w_in = {n: din(n, [DIM, 128], dt.float32r)
        for n in ("wq0", "wq1", "wk0", "wk1", "wv0", "wv1")}
    b_in = {n: din(n, [128, 1]) for n in ("bq0", "bq1", "bk0", "bk1", "bv0", "bv1")}
    g_in = {
        n: din(n, [128, 1])
        for n in ("gq0", "gq0s", "gq1", "gq1s", "gk0", "gk0s", "gk1", "gk1s")
    }
    cosq = din("cosq", [128, S])
    sinq = din("sinq", [128, S])
    cosq1 = din("cosq1", [128, HALF])
    sinq1 = din("sinq1", [128, HALF])
    kc_in = [din("kc0", [128, CACHED], dt.float32r),
         din("kc1", [128, CACHED], dt.float32r)]
    vc_in = [din("vc0", [CACHED, 128], dt.bfloat16), din("vc1", [CACHED, 128], dt.bfloat16)]
    wo_in = [din("wo0", [128, DIM], dt.float32r),
         din("wo1", [128, DIM], dt.float32r)]
    bo128 = din("bo128", [128, DIM])
    masks = {n: din(n, [1, 1]) for n in ("mk1", "mq3", "mq4", "rnsel", "rnsel2")}
    out_ap = nc.dram_tensor("out", [OUT_ROWS, DIM], dt.float32,
                            kind="ExternalOutput").ap()

    with tile.TileContext(nc) as tc:
        with (
            tc.tile_pool(name="persist", bufs=1) as persist,
            tc.tile_pool(name="dram", bufs=1, space="DRAM") as dram,
        ):
            # ---------- persistent tiles ----------
            kT = [persist.tile([128, KEYS], dt.float32r, tag=f"kT{i}", name=f"kT{i}")
                  for i in range(2)]
            qT0 = persist.tile([128, S], dt.float32r, tag="qT0")
            qT1 = persist.tile([128, HALF], dt.float32r, tag="qT1")
            v_sb = [persist.tile([128, N_KT, 128], dt.bfloat16, tag=f"v{i}", name=f"v{i}")
                    for i in range(2)]
            wo_r = [persist.tile([128, DIM], dt.float32r, tag=f"wo{i}", name=f"wo{i}")
                    for i in range(2)]
            ones_r = persist.tile([128, 1], dt.float32r, tag="ones_r")
            ones_b = persist.tile([128, 1], dt.bfloat16, tag="ones_b")
            ident = persist.tile([128, 128], dt.float32, tag="ident")
            m_ap = {}
            for n in masks:
                t = persist.tile([1, 1], dt.float32, tag=f"m_{n}", name=f"m_{n}")
                nc.sync.dma_start(out=t, in_=masks[n])
                m_ap[n] = t

            make_identity(nc, ident)
            tmp1 = persist.tile([128, 1], dt.float32, tag="tmp1")
            nc.vector.memset(tmp1, 1.0)
            nc.vector.tensor_copy(ones_r, tmp1)
            nc.vector.memset(ones_b, 1.0)

            # cached V -> v tiles (early; overlaps with proj compute)
            for i in range(2):
                nc.sync.dma_start(
                    out=v_sb[i][:, 0:19, :],
                    in_=vc_in[i][0:2432, :].rearrange("(t p) d -> p t d", p=128),
                )
                nc.sync.dma_start(out=v_sb[i][0:80, 19, :], in_=vc_in[i][2432:2512, :])

            ssq_in = dram.tile([8, S], dt.float32, tag="ssq_in")
            ssq_out = dram.tile([8, S], dt.float32, tag="ssq_out")

            # ---------- phase 1: projections + rope ----------
            with (
                tc.tile_pool(name="bgp", bufs=1) as bgp,
                tc.tile_pool(name="xstage", bufs=2) as xs,
                tc.tile_pool(name="xr", bufs=2) as xrp,
                tc.tile_pool(name="praw", bufs=3) as prp,
                tc.tile_pool(name="ptmp", bufs=2) as ptp,
                tc.tile_pool(name="ssqs", bufs=3) as ssqsb,
                tc.tile_pool(name="p1ps", bufs=4, space="PSUM") as p1ps,
                tc.tile_pool(name="ssqps", bufs=2, space="PSUM") as ssqps,
                tc.tile_pool(name="tps", bufs=2, space="PSUM") as tps,
            ):
                b_ap = {}
                for n, src in list(b_in.items()) + list(g_in.items()):
                    t = bgp.tile([128, 1], dt.float32, tag=f"b_{n}", name=f"b_{n}")
                    nc.sync.dma_start(out=t, in_=src)
                    b_ap[n] = t

                def proj_chunk(xsrc, soff, ssz, specs, ctab_d, stab_d):
                    xr = xrp.tile([128, CT, 256], dt.float32r, tag="xr")
                    nc.sync.dma_start(
                        out=xr[:, :, 0:ssz],
                        in_=xsrc.rearrange("(ct p) s -> p ct s", p=128)[
                            :, :, soff:soff + ssz],
                    )
                    ctab = ptp.tile([128, 256], dt.float32, tag="ctab")
                    stab = ptp.tile([128, 256], dt.float32, tag="stab")
                    nc.sync.dma_start(out=ctab[:, 0:ssz],
                                      in_=ctab_d[:, soff:soff + ssz])
                    nc.sync.dma_start(out=stab[:, 0:ssz],
                                      in_=stab_d[:, soff:soff + ssz])
                    for spec in specs:
                        kind, wname, bname = spec[:3]
                        ps = p1ps.tile([128, 256], dt.float32, tag="projps")
                        for ct in range(CT):
                            nc.tensor.matmul(ps[:, 0:ssz], w_r[wname][:, ct],
                                             xr[:, ct, 0:ssz],
                                             start=(ct == 0), stop=(ct == CT - 1))
                        pr = prp.tile([128, 256], dt.float32, tag="praw")
                        nc.vector.tensor_scalar(
                            out=pr[:, 0:ssz], in0=ps[:, 0:ssz],
                            scalar1=b_ap[bname], scalar2=None, op0=OP.add)
                        if kind == "v":
                            vi = spec[3]
                            for bo_ in range(0, ssz, 128):
                                bsz = min(128, ssz - bo_)
                                ptile = tps.tile([128, 128], dt.float32, tag="tp")
                                nc.tensor.transpose(
                                    ptile[0:bsz, :], pr[:, bo_:bo_ + bsz], ident)
                                ti = 20 + (soff + bo_) // 128
                                nc.vector.tensor_copy(
                                    v_sb[vi][0:bsz, ti, :], ptile[0:bsz, :])
                        else:
                            rows, gname, gsname, dest = spec[3:]
                            sq = ptp.tile([128, 256], dt.float32r, tag="sq")
                            nc.vector.tensor_mul(sq[:, 0:ssz], pr[:, 0:ssz],
                                                 pr[:, 0:ssz])
                            sps = ssqps.tile([1, 256], dt.float32, tag="ssqps")
                            nc.tensor.matmul(sps[0:1, 0:ssz], ones_r, sq[:, 0:ssz],
                                             start=True, stop=True)
                            for row, mask in rows:
                                srow = ssqsb.tile([1, 256], dt.float32, tag="ssq")
                                nc.vector.tensor_scalar(
                                    out=srow[0:1, 0:ssz],
                                    in0=sps[0:1, 0:ssz],
                                    scalar1=(m_ap[mask] if mask else 1.0),
                                    scalar2=None, op0=OP.mult)
                                nc.sync.dma_start(
                                    out=ssq_in[row:row + 1, soff:soff + ssz],
                                    in_=srow[0:1, 0:ssz])
                            # rope: dest = (pr*g) . cos + (swap(pr)*gs) . sin
                            sh = ptp.tile([128, 256], dt.float32, tag="sh")
                            nc.sync.dma_start(out=sh[0:64, 0:ssz],
                                              in_=pr[64:128, 0:ssz])
                            nc.sync.dma_start(out=sh[64:128, 0:ssz],
                                              in_=pr[0:64, 0:ssz])
                            tc_ = ptp.tile([128, 256], dt.float32, tag="tcos")
                            nc.vector.scalar_tensor_tensor(
                                out=tc_[:, 0:ssz], in0=pr[:, 0:ssz],
                                scalar=b_ap[gname],
                                in1=ctab[:, 0:ssz],
                                op0=OP.mult, op1=OP.mult)
                            ts_ = ptp.tile([128, 256], dt.float32, tag="tsin")
                            nc.vector.scalar_tensor_tensor(
                                out=ts_[:, 0:ssz], in0=sh[:, 0:ssz],
                                scalar=b_ap[gsname],
                                in1=stab[:, 0:ssz],
                                op0=OP.mult, op1=OP.mult)
                            nc.vector.tensor_add(dest[:, soff:soff + ssz],
                                                 tc_[:, 0:ssz], ts_[:, 0:ssz])

                # --- main S loop (slot0 q/k/v + slot1 k/v) ---
                with (
                    tc.tile_pool(name="wts", bufs=1) as wpool,
                    tc.tile_pool(name="wstage", bufs=1) as ws,
                ):
                    w_r = {}
                    for n in ("wq0", "wk0", "wv0", "wk1", "wv1"):
                        wr = wpool.tile([128, CT, 128], dt.float32r, tag=f"w_{n}")
                        nc.sync.dma_start(
                            out=wr, in_=w_in[n].rearrange("(ct p) f -> p ct f", p=128))
                        w_r[n] = wr

                    for soff, ssz in S_CHUNKS:
                        proj_chunk(
                            xT, soff, ssz,
                            [
                                ("qk", "wq0", "bq0", [(2, None)], "gq0", "gq0s", qT0),
                                ("qk", "wk0", "bk0", [(0, None)], "gk0", "gk0s",
                                 kT[0][:, CACHED:]),
                                ("v", "wv0", "bv0", 0),
                                ("qk", "wk1", "bk1", [(1, "mk1")], "gk1", "gk1s",
                                 kT[1][:, CACHED:]),
                                ("v", "wv1", "bv1", 1),
                            ],
                            cosq, sinq,
                        )

                # --- slot1 q loop ---
                with (
                    tc.tile_pool(name="wts1", bufs=1) as wpool1,
                    tc.tile_pool(name="wstage1", bufs=1) as ws1,
                ):
                    wq1r = wpool1.tile([128, CT, 128], dt.float32r, tag="w_wq1")
                    nc.sync.dma_start(
                        out=wq1r, in_=w_in["wq1"].rearrange("(ct p) f -> p ct f", p=128))
                    w_r["wq1"] = wq1r
                    for soff, ssz in H_CHUNKS:
                        proj_chunk(
                            xu, soff, ssz,
                            [("qk", "wq1", "bq1", [(3, "mq3"), (4, "mq4")],
                              "gq1", "gq1s", qT1)],
                            cosq1, sinq1,
                        )

            # ---------- ssq AllReduce (issued before V so it overlaps) ----------
            nc.gpsimd.collective_compute(
                "AllReduce", OP.add,
                replica_groups=[list(range(N_CORES))],
                ins=[ssq_in.opt()], outs=[ssq_out.opt()],
            )

            # ---------- cached K load (round to f32r) ----------
            for i in range(2):
                nc.sync.dma_start(out=kT[i][:, 0:CACHED], in_=kc_in[i])
                nc.sync.dma_start(out=wo_r[i], in_=wo_in[i])
            with tc.tile_pool(name="normp", bufs=1) as npo:
                eps_t = npo.tile([1, 1], dt.float32, tag="eps")
                nc.vector.memset(eps_t, EPS)
                rn_d = dram.tile([3, S], dt.float32, tag="rn_d")
                sc1 = npo.tile([1, S], dt.float32, tag="sc1")
                sc2 = npo.tile([1, S], dt.float32, tag="sc2")
                sc3 = npo.tile([1, S], dt.float32, tag="sc3")
                bb = npo.tile([128, S], dt.float32, tag="bb")
                eps128 = npo.tile([128, 1], dt.float32, tag="eps128")
                nc.vector.memset(eps128, EPS)
                sq_d = dram.tile([2, S], dt.float32, tag="sq_d")

                def rsqrt_via_reshape(src_sc, dst_row, drow):
                    # src_sc [1,S] sum-of-squares -> rn_d[dst_row] = rsqrt(v+eps)
                    nc.sync.dma_start(out=sq_d[drow:drow + 1, :], in_=src_sc)
                    rsh = npo.tile([26, 120], dt.float32, tag="rsh", bufs=2)
                    nc.sync.dma_start(
                        out=rsh, in_=sq_d[drow:drow + 1, :].rearrange(
                            "o (t p) -> (o t) p", p=120))
                    rsh2 = npo.tile([26, 120], dt.float32, tag="rsh2", bufs=2)
                    nc.scalar.activation(out=rsh2, in_=rsh, func=AF.Sqrt,
                                         bias=eps128[0:26], scale=1.0 / DIM)
                    nc.vector.reciprocal(rsh, rsh2)
                    nc.sync.dma_start(
                        out=rn_d[dst_row:dst_row + 1, :].rearrange(
                            "o (t p) -> (o t) p", p=120),
                        in_=rsh)

                # k norm
                r0 = npo.tile([1, S], dt.float32, tag="ssqr", bufs=3)
                nc.sync.dma_start(out=r0, in_=ssq_out[0:1, :])
                r1 = npo.tile([1, S], dt.float32, tag="ssqr", bufs=3)
                nc.sync.dma_start(out=r1, in_=ssq_out[1:2, :])
                nc.vector.tensor_add(sc1, r0, r1)
                rsqrt_via_reshape(sc1, 0, 0)
                nc.scalar.dma_start(out=bb,
                                    in_=rn_d[0:1, :].to_broadcast((128, S)))
                nc.vector.tensor_mul(kT[0][:, CACHED:], kT[0][:, CACHED:], bb)
                nc.vector.tensor_mul(kT[1][:, CACHED:], kT[1][:, CACHED:], bb)
                # q norm
                r2 = npo.tile([1, S], dt.float32, tag="ssqr", bufs=3)
                nc.sync.dma_start(out=r2, in_=ssq_out[2:3, :])
                r3 = npo.tile([1, S], dt.float32, tag="ssqr", bufs=3)
                nc.sync.dma_start(out=r3, in_=ssq_out[3:4, :])
                r4 = npo.tile([1, S], dt.float32, tag="ssqr", bufs=3)
                nc.sync.dma_start(out=r4, in_=ssq_out[4:5, :])
                nc.vector.tensor_add(sc1[:, 0:HALF], r2[:, 0:HALF], r3[:, 0:HALF])
                nc.vector.tensor_add(sc1[:, HALF:], r2[:, HALF:], r4[:, 0:HALF])
                rsqrt_via_reshape(sc1, 1, 1)
                sc3_d = rn_d  # rnq now in rn_d[1]
                nc.sync.dma_start(out=sc3, in_=rn_d[1:2, :])
                bb2 = npo.tile([128, S], dt.float32, tag="bb")
                nc.sync.dma_start(out=bb2,
                                  in_=rn_d[1:2, :].to_broadcast((128, S)))
                nc.vector.tensor_mul(qT0, qT0, bb2)
                # slot1 q norm select
                nc.vector.tensor_scalar(out=sc2[:, 0:HALF], in0=sc3[:, 0:HALF],
                                        scalar1=m_ap["rnsel"], scalar2=None,
                                        op0=OP.mult)
                nc.vector.tensor_scalar(out=sc2[:, HALF:], in0=sc3[:, HALF:],
                                        scalar1=m_ap["rnsel2"], scalar2=None,
                                        op0=OP.mult)
                nc.vector.tensor_add(sc1[:, 0:HALF], sc2[:, 0:HALF], sc2[:, HALF:])
                nc.sync.dma_start(out=rn_d[2:3, 0:HALF], in_=sc1[:, 0:HALF])
                bb3 = npo.tile([128, HALF], dt.float32, tag="bbh")
                nc.sync.dma_start(out=bb3,
                                  in_=rn_d[2:3, 0:HALF].to_broadcast((128, HALF)))
                nc.vector.tensor_mul(qT1, qT1, bb3)

            # ---------- phase 2: attention + o-proj ----------
            partial1 = dram.tile([S, DIM], dt.float16, tag="partial1")
            partial2 = dram.tile([HALF, DIM], dt.float16, tag="partial2")

            with (
                tc.tile_pool(name="stps", bufs=2, space="PSUM") as stps_p,
                tc.tile_pool(name="pvps", bufs=2, space="PSUM") as pvps_p,
                tc.tile_pool(name="rsps", bufs=2, space="PSUM") as rsps_p,
                tc.tile_pool(name="ptp2", bufs=6) as ptp2,
                tc.tile_pool(name="att_sb", bufs=3) as asb,
            ):
                def attention(slot, qchunks, qT_t, part_dram):
                    for qoff, qsz in qchunks:
                        pvps = pvps_p.tile([128, 512], dt.float32, tag="pv")
                        rsps = rsps_p.tile([128, 512], dt.float32, tag="rsop")
                        for g in EXP_GROUPS:
                            stp = stps_p.tile([128, 2, 512], dt.float32, tag="st")
                            for j, t in enumerate(g):
                                koff, ksz = KEY_TILES[t]
                                nc.tensor.matmul(
                                    stp[0:ksz, j, 0:qsz],
                                    kT[slot][:, koff:koff + ksz],
                                    qT_t[:, qoff:qoff + qsz],
                                    start=True, stop=True)
                            pt = ptp2.tile([128, 2, 512], dt.bfloat16, tag="pt")
                            gsz = KEY_TILES[g[0]][1] if len(g) == 1 else 128
                            nc.scalar.activation(
                                out=pt[0:gsz, 0:len(g), 0:qsz],
                                in_=stp[0:gsz, 0:len(g), 0:qsz],
                                func=AF.Exp, bias=0.0, scale=SCALE)
                            for j, t in enumerate(g):
                                koff, ksz = KEY_TILES[t]
                                nc.tensor.matmul(
                                    pvps[:, 0:qsz], v_sb[slot][0:ksz, t, :],
                                    pt[0:ksz, j, 0:qsz],
                                    start=(t == 0), stop=(t == N_KT - 1))
                                nc.tensor.matmul(
                                    rsps[0:1, 0:qsz], ones_b[0:ksz, :],
                                    pt[0:ksz, j, 0:qsz],
                                    start=(t == 0), stop=(t == N_KT - 1))
                        rc = asb.tile([1, 512], dt.float32, tag="rc")
                        nc.vector.tensor_copy(rc[:, 0:qsz], rsps[0:1, 0:qsz])
                        rc_d = dram.tile([1, 512], dt.float32, tag="rc_d")
                        nc.sync.dma_start(out=rc_d[:, 0:qsz], in_=rc[:, 0:qsz])
                        rsb = asb.tile([128, 512], dt.float32, tag="rsb")
                        nc.sync.dma_start(
                            out=rsb[:, 0:qsz],
                            in_=rc_d[0:1, 0:qsz].to_broadcast((128, qsz)))
                        rcb = asb.tile([128, 512], dt.float32, tag="rcb")
                        nc.vector.reciprocal(rcb[:, 0:qsz], rsb[:, 0:qsz])
                        oT = asb.tile([128, 512], dt.float32r, tag="oT")
                        nc.vector.tensor_mul(oT[:, 0:qsz], pvps[:, 0:qsz],
                                             rcb[:, 0:qsz])
                        for stoff in range(0, qsz, 128):
                            stsz = min(128, qsz - stoff)
                            for ec in range(3):
                                ops = rsps_p.tile([128, 512], dt.float32, tag="rsop")
                                nc.tensor.matmul(
                                    ops[0:stsz, :], oT[:, stoff:stoff + stsz],
                                    wo_r[slot][:, ec * 512:(ec + 1) * 512],
                                    start=True, stop=True)
                                ev = asb.tile([128, 512], dt.float16, tag="ev")
                                nc.any.tensor_copy(ev[0:stsz, :], ops[0:stsz, :])
                                nc.sync.dma_start(
                                    out=part_dram[qoff + stoff:qoff + stoff + stsz,
                                                  ec * 512:(ec + 1) * 512],
                                    in_=ev[0:stsz, :])

                rsA = dram.tile([OUT_ROWS, DIM], dt.float16, tag="rsA")
                rsB = dram.tile([OUT_ROWS, DIM], dt.float16, tag="rsB")
                attention(0, Q_CHUNKS0, qT0, partial1)
                # RS for slot0 partials overlaps slot1 attention
                nc.gpsimd.collective_compute(
                    "ReduceScatter", OP.add,
                    replica_groups=[list(range(N_CORES))],
                    ins=[partial1.opt()], outs=[rsA.opt()],
                )
                attention(1, Q_CHUNKS1, qT1, partial2)
                nc.gpsimd.collective_compute(
                    "ReduceScatter", OP.add,
                    replica_groups=[[0, 1, 2, 3], [4, 5, 6, 7]],
                    ins=[partial2.opt()], outs=[rsB.opt()],
                )
            with tc.tile_pool(name="fin", bufs=2) as fin:
                bo_sb = fin.tile([128, DIM], dt.float32, tag="bo")
                nc.sync.dma_start(out=bo_sb, in_=bo128)
                for roff in range(0, OUT_ROWS, 128):
                    rsz = min(128, OUT_ROWS - roff)
                    ta = fin.tile([128, DIM], dt.float16, tag="fa")
                    tb = fin.tile([128, DIM], dt.float16, tag="fb")
                    tf = fin.tile([128, DIM], dt.float32, tag="ff")
                    nc.sync.dma_start(out=ta[0:rsz, :], in_=rsA[roff:roff + rsz, :])
                    nc.sync.dma_start(out=tb[0:rsz, :], in_=rsB[roff:roff + rsz, :])
                    nc.vector.tensor_add(tf[0:rsz, :], ta[0:rsz, :], tb[0:rsz, :])
                    nc.vector.tensor_add(tf[0:rsz, :], tf[0:rsz, :], bo_sb[0:rsz, :])
                    nc.sync.dma_start(out=out_ap[roff:roff + rsz, :],
                                      in_=tf[0:rsz, :])

    nc.compile()
    _BUILD_CACHE["nc"] = nc
    return nc


PERM = np.concatenate([np.arange(0, 128, 2), np.arange(1, 128, 2)])
PERM_SW = np.concatenate([PERM[64:], PERM[:64]])


def _host_prep(inputs):
    x = np.asarray(inputs["x"])[0]  # [S, DIM]
    theta = np.asarray(inputs["freqs_theta"])
    cache_k = np.asarray(inputs["cache_k"])[0]  # [L, 12, 128]
    cache_v = np.asarray(inputs["cache_v"])[0]
    wq, wk, wv, wo = (np.asarray(inputs[n]) for n in ("wq", "wk", "wv", "wo"))
    bq, bk, bv, bo = (np.asarray(inputs[n]) for n in ("bq", "bk", "bv", "bo"))
    gq, gk = np.asarray(inputs["gq"]), np.asarray(inputs["gk"])
    f, h, w = int(inputs["grid_f"]), int(inputs["grid_h"]), int(inputs["grid_w"])
    current_start = int(inputs["current_start"])
    global_end = int(inputs["global_end_index"])
    local_end_in = int(inputs["local_end_index"])

    frame_seqlen = h * w
    start_frame = current_start // frame_seqlen
    current_end = current_start + S
    local_end = local_end_in + current_end - global_end
    local_start = local_end - S
    win_start = max(0, local_end - MAX_ATTN)
    assert local_start - win_start == CACHED, (win_start, local_start)

    # rope angle table [S, 64]
    c = HD // 2
    ct_ = c - 2 * (c // 3)  # 22
    ch_ = c // 3  # 21
    ang = np.concatenate([
        np.broadcast_to(theta[start_frame:start_frame + f, :ct_][:, None, None, :],
                        (f, h, w, ct_)),
        np.broadcast_to(theta[:h, ct_:ct_ + ch_][None, :, None, :], (f, h, w, ch_)),
        np.broadcast_to(theta[:w, ct_ + ch_:ct_ + 2 * ch_][None, None, :, :],
                        (f, h, w, ch_)),
    ], axis=-1).reshape(S, c)
    cosT = np.cos(ang).T.astype(np.float32)  # [64, S]
    sinT = np.sin(ang).T.astype(np.float32)
    cosD = np.ascontiguousarray(np.concatenate([cosT, cosT], 0))  # [128, S]
    sinD = np.ascontiguousarray(np.concatenate([-sinT, sinT], 0))

    xTf = np.ascontiguousarray(x.T, np.float32)  # [DIM, S]

    def wslice(wm, head, perm):
        block = wm[head * HD:(head + 1) * HD, :][perm, :]  # [128, DIM]
        return np.ascontiguousarray(block.T, np.float32)  # [DIM, 128]

    def col(vec, head, perm):
        return np.ascontiguousarray(
            vec[head * HD:(head + 1) * HD][perm][:, None], np.float32)

    in_maps = []
    for cidx in range(N_CORES):
        h0 = cidx
        h1 = 8 + (cidx % 4)
        half = cidx // 4
        hsl = slice(half * HALF, (half + 1) * HALF)
        kc = []
        vc = []
        for hh in (h0, h1):
            arr = cache_k[win_start:local_start, hh, :]  # [CACHED, 128]
            kc.append(np.ascontiguousarray(arr.T[PERM], np.float32))
            vc.append(np.ascontiguousarray(
                cache_v[win_start:local_start, hh, :]).astype(ml_dtypes.bfloat16))
        iden = np.arange(128)
        m = {
            "xT": xTf,
            "xu": np.ascontiguousarray(xTf[:, hsl]),
            "wq0": wslice(wq, h0, PERM), "wq1": wslice(wq, h1, PERM),
            "wk0": wslice(wk, h0, PERM), "wk1": wslice(wk, h1, PERM),
            "wv0": wslice(wv, h0, iden), "wv1": wslice(wv, h1, iden),
            "bq0": col(bq, h0, PERM), "bq1": col(bq, h1, PERM),
            "bk0": col(bk, h0, PERM), "bk1": col(bk, h1, PERM),
            "bv0": col(bv, h0, iden), "bv1": col(bv, h1, iden),
            "gq0": col(gq, h0, PERM), "gq0s": col(gq, h0, PERM_SW),
            "gq1": col(gq, h1, PERM), "gq1s": col(gq, h1, PERM_SW),
            "gk0": col(gk, h0, PERM), "gk0s": col(gk, h0, PERM_SW),
            "gk1": col(gk, h1, PERM), "gk1s": col(gk, h1, PERM_SW),
            "cosq": cosD, "sinq": sinD,
            "cosq1": np.ascontiguousarray(cosD[:, hsl]),
            "sinq1": np.ascontiguousarray(sinD[:, hsl]),
            "kc0": kc[0], "kc1": kc[1], "vc0": vc[0], "vc1": vc[1],
            "wo0": np.ascontiguousarray(wo[:, h0 * HD:(h0 + 1) * HD].T, np.float32),
            "wo1": np.ascontiguousarray(wo[:, h1 * HD:(h1 + 1) * HD].T, np.float32),
            "bo128": np.broadcast_to(bo[None, :], (128, DIM)).astype(np.float32).copy(),
            "mk1": np.full((1, 1), 1.0 if cidx < 4 else 0.0, np.float32),
            "mq3": np.full((1, 1), 1.0 if half == 0 else 0.0, np.float32),
            "mq4": np.full((1, 1), 1.0 if half == 1 else 0.0, np.float32),
            "rnsel": np.full((1, 1), 1.0 if half == 0 else 0.0, np.float32),
            "rnsel2": np.full((1, 1), 1.0 if half == 1 else 0.0, np.float32),
        }
        in_maps.append(m)
    return in_maps


def kernel(**inputs):
    from concourse.bass_utils import run_bass_kernel_spmd

    nc = _build()
    in_maps = _host_prep(inputs)
    res = run_bass_kernel_spmd(nc, in_maps, list(range(N_CORES)))
    out = np.concatenate([res.results[i]["out"] for i in range(N_CORES)], axis=0)
    return out[None].astype(np.float32)
